# revision 20
# baseline (speedup 1.0000x reference)
"""DA-HGNN TRN2 Bass kernel: 8-core row-sharded SPMD implementation.

Self-contained: hardcodes shapes n=4096, F=512, hid=256, d=8, heads=4.
kernel(**inputs) takes full unsharded inputs, returns full (4096, 8) f32.

Math notes vs reference.py:
- edge/H/xx products use fp8 (operands are exactly 0/1; fp32 PSUM accumulate
  => bit-exact counts), with DoubleRow perf mode.
- softmax is shift-invariant per row, so the rho_t shift has no effect on the
  output when rho is non-constant.  When rho is constant the reference (on the
  neuron backend) yields elu(NaN)=0 rows; we reproduce that exactly with a
  multiplicative flag (1 - [max(rho)==min(rho)]) applied after elu.  No NaNs
  ever materialize on the device.
"""
from contextlib import ExitStack
import numpy as np
import ml_dtypes

import concourse.bass as bass
import concourse.bacc as bacc
import concourse.mybir as mybir
import concourse.tile as tile
from concourse.bass_utils import run_bass_kernel_spmd

F32 = mybir.dt.float32
BF16 = mybir.dt.bfloat16
FP8 = mybir.dt.float8e4
U32 = mybir.dt.uint32
AF = mybir.ActivationFunctionType
OP = mybir.AluOpType
AX = mybir.AxisListType
DR = mybir.MatmulPerfMode.DoubleRow

P = 8
N = 4096
F = 512
HID = 256
D = 8
R = N // P        # 512
RC = R // 128     # 4
NC = N // 128     # 32
FC = F // 128     # 4
HC = HID // 128   # 2
NCC = N // 512    # 8
SIGMA = 0.5


def build():
    nc = bacc.Bacc(None, num_devices=P)

    XT = nc.declare_dram_parameter("XT", [128, FC, N], BF16, isOutput=False)
    XTk = nc.declare_dram_parameter("XTk", [128, FC, R], BF16, isOutput=False)
    Xn = nc.declare_dram_parameter("Xn", [128, NC, F], BF16, isOutput=False)
    theta_t = nc.declare_dram_parameter("theta_t", [128, FC, HID], BF16, isOutput=False)
    Wcat = nc.declare_dram_parameter("Wcat", [128, HC, 40], BF16, isOutput=False)
    wvx = nc.declare_dram_parameter("wvx", [128, HC, 8], BF16, isOutput=False)
    wve = nc.declare_dram_parameter("wve", [128, HC, 10], BF16, isOutput=False)
    wvl = nc.declare_dram_parameter("wvl", [128, HC, 2], BF16, isOutput=False)
    linw = nc.declare_dram_parameter("linw", [32, HID], BF16, isOutput=False)
    linb = nc.declare_dram_parameter("linb", [128, HC], F32, isOutput=False)
    ones_row = nc.declare_dram_parameter("ones_row", [1, 128], F32, isOutput=False)
    ones_col = nc.declare_dram_parameter("ones_col", [128, 1], F32, isOutput=False)
    ident = nc.declare_dram_parameter("ident", [128, 128], BF16, isOutput=False)
    selbig = nc.declare_dram_parameter("selbig", [128, RC, N], BF16, isOutput=False)
    roff_in = nc.declare_dram_parameter("roff_in", [1, 1], U32, isOutput=False)

    out_ext = nc.declare_dram_parameter("out", [8, N], F32, isOutput=True)

    rg = [list(range(P))]
    d_sq = nc.dram_tensor("d_sq", [1, N], F32)
    d_rad_i = nc.dram_tensor("d_rad_i", [1, 1], F32)
    d_rad_o = nc.dram_tensor("d_rad_o", [P, 1], F32, addr_space="Shared")
    d_edge_i = nc.dram_tensor("d_edge_i", [R, N], FP8)
    d_edge = nc.dram_tensor("d_edge", [N, N], FP8, addr_space="Shared")
    d_H_i = nc.dram_tensor("d_H_i", [R, N], FP8)
    d_H = nc.dram_tensor("d_H", [N, N], FP8, addr_space="Shared")
    d_Hk = nc.dram_tensor("d_Hk", [R, N], BF16)
    d_Dv_i = nc.dram_tensor("d_Dv_i", [R, 1], F32)
    d_Dv = nc.dram_tensor("d_Dv", [N, 1], F32, addr_space="Shared")
    d_En_i = nc.dram_tensor("d_En_i", [R, HID], BF16)
    d_En = nc.dram_tensor("d_En", [N, HID], BF16, addr_space="Shared")
    d_v1_i = nc.dram_tensor("d_v1_i", [9, R], F32)
    d_v1 = nc.dram_tensor("d_v1", [9 * P, R], F32, addr_space="Shared")
    d_v2_i = nc.dram_tensor("d_v2_i", [1, R], F32)
    d_v2 = nc.dram_tensor("d_v2", [P, R], F32, addr_space="Shared")
    d_XhT_i = nc.dram_tensor("d_XhT_i", [HID, R], BF16)
    d_XhT = nc.dram_tensor("d_XhT", [HID * P, R], BF16, addr_space="Shared")
    d_rho1_i = nc.dram_tensor("d_rho1_i", [R, 1], F32)
    d_rho1 = nc.dram_tensor("d_rho1", [N, 1], F32, addr_space="Shared")
    d_re1_i = nc.dram_tensor("d_re1_i", [R, 1], F32)
    d_re1 = nc.dram_tensor("d_re1", [N, 1], F32, addr_space="Shared")
    d_rho2_i = nc.dram_tensor("d_rho2_i", [R, 1], F32)
    d_rho2 = nc.dram_tensor("d_rho2", [N, 1], F32, addr_space="Shared")
    d_re2_i = nc.dram_tensor("d_re2_i", [R, 1], F32)
    d_re2 = nc.dram_tensor("d_re2", [N, 1], F32, addr_space="Shared")
    d_p1_i = nc.dram_tensor("d_p1_i", [32, N], F32)
    d_p1 = nc.dram_tensor("d_p1", [32, N], F32, addr_space="Shared")
    d_p2_i = nc.dram_tensor("d_p2_i", [32, N], F32)
    d_p2 = nc.dram_tensor("d_p2", [32, N], F32, addr_space="Shared")
    d_p3_i = nc.dram_tensor("d_p3_i", [8, N], F32)
    d_p3 = nc.dram_tensor("d_p3", [8, N], F32, addr_space="Shared")
    d_p4_i = nc.dram_tensor("d_p4_i", [8, N], F32)
    d_p4 = nc.dram_tensor("d_p4", [8, N], F32, addr_space="Shared")
    d_EnT = nc.dram_tensor("d_EnT", [32, N], BF16)
    d_E2T = nc.dram_tensor("d_E2T", [8, N], BF16)
    d_XlT = nc.dram_tensor("d_XlT", [HID, N], BF16)
    d_in1 = nc.dram_tensor("d_in1", [1, N], F32)
    d_in2 = nc.dram_tensor("d_in2", [1, N], F32)

    with tile.TileContext(nc) as tc, ExitStack() as _stk:
        pers = _stk.enter_context(tc.tile_pool(name="pers", bufs=1))

        onr = pers.tile([1, 128], F32, tag="onr")
        onc = pers.tile([128, 1], F32, tag="onc")
        idt = pers.tile([128, 128], BF16, tag="idt")
        nc.sync.dma_start(onr[:], ones_row[:])
        nc.sync.dma_start(onc[:], ones_col[:])
        nc.sync.dma_start(idt[:], ident[:])

        roff = nc.gpsimd.alloc_register("roff")
        nc.gpsimd.reg_load(roff, roff_in[0:1, 0:1])
        roffs = nc.gpsimd.snap(roff)

        wvx_t = pers.tile([128, HC, 8], BF16, tag="wvx_t")
        wve_t = pers.tile([128, HC, 10], BF16, tag="wve_t")
        wvl_t = pers.tile([128, HC, 2], BF16, tag="wvl_t")
        wcat_t = pers.tile([128, HC, 40], BF16, tag="wcat_t")
        nc.sync.dma_start(wvx_t[:], wvx[:])
        nc.sync.dma_start(wve_t[:], wve[:])
        nc.sync.dma_start(wvl_t[:], wvl[:])
        nc.sync.dma_start(wcat_t[:], Wcat[:])

        XhkT = pers.tile([128, HC, R], BF16, tag="XhkT")
        EkT = pers.tile([128, HC, R], BF16, tag="EkT")
        Dv_all = pers.tile([128, NC], F32, tag="Dv_all")
        Dv_col = pers.tile([128, RC], F32, tag="Dv_col")

        # ---------- phase 1: sq / Gram / dist / radius / edge ----------
        with tc.tile_pool(name="p1a", bufs=1) as p1a:
            xt = p1a.tile([128, FC, N], BF16, tag="xt")
            xtk = p1a.tile([128, FC, R], BF16, tag="xtk")
            nc.sync.dma_start(xt[:], XT[:])
            nc.sync.dma_start(xtk[:], XTk[:])
            sq_row = p1a.tile([1, N], F32, tag="sq_row")

            with tc.tile_pool(name="p1sq", bufs=2) as p1sq, \
                 tc.tile_pool(name="ps_sq", bufs=1, space="PSUM") as ps_sq:
                pqs = [ps_sq.tile([1, 512], F32, tag=f"pq{c}", name=f"pq{c}") for c in range(NCC)]
                for kc in range(FC):
                    x2 = p1sq.tile([128, N], F32, tag="x2")
                    nc.vector.scalar_tensor_tensor(
                        out=x2[:], in0=xt[:, kc, :], scalar=1.0,
                        in1=xt[:, kc, :], op0=OP.mult, op1=OP.mult)
                    for c in range(NCC):
                        nc.tensor.matmul(pqs[c][:], onc[:], x2[:, c * 512:(c + 1) * 512],
                                         start=(kc == 0), stop=(kc == FC - 1))
                for c in range(NCC):
                    nc.vector.tensor_copy(sq_row[0:1, c * 512:(c + 1) * 512], pqs[c][:])

            sq_col = p1a.tile([128, RC], F32, tag="sq_col")
            nc.sync.dma_start(d_sq[:], sq_row[:])
            nc.gpsimd.dma_start(
                sq_col[:],
                d_sq[0:1, bass.ds(roffs, R)].rearrange("x (a b) -> x b a", a=RC))

            with tc.tile_pool(name="p1w", bufs=3) as p1w, \
                 tc.tile_pool(name="p1b", bufs=1) as p1b, \
                 tc.tile_pool(name="ps1g", bufs=3, space="PSUM") as ps1g, \
                 tc.tile_pool(name="ps1b", bufs=2, space="PSUM") as ps1b:
                sqb = p1b.tile([128, N], F32, tag="sqb")
                for c in range(NCC):
                    pb = ps1b.tile([128, 512], F32, tag="pb")
                    nc.tensor.matmul(pb[:], onr[:], sq_row[0:1, c * 512:(c + 1) * 512],
                                     start=True, stop=True)
                    nc.vector.tensor_copy(sqb[:, c * 512:(c + 1) * 512], pb[:])

                dist = p1b.tile([128, RC, N], BF16, tag="dist")
                radacc = p1b.tile([128, 32], F32, tag="radacc")
                for mi in range(RC):
                    for c in range(NCC):
                        pg = ps1g.tile([128, 512], F32, tag="pg")
                        for kc in range(FC):
                            nc.tensor.matmul(pg[:], xtk[:, kc, mi * 128:(mi + 1) * 128],
                                             xt[:, kc, c * 512:(c + 1) * 512],
                                             start=(kc == 0), stop=(kc == FC - 1))
                        td = p1w.tile([128, 512], F32, tag="td")
                        nc.vector.scalar_tensor_tensor(
                            out=td[:], in0=pg[:], scalar=-2.0,
                            in1=sqb[:, c * 512:(c + 1) * 512], op0=OP.mult, op1=OP.add)
                        nc.scalar.activation(
                            dist[:, mi, c * 512:(c + 1) * 512], td[:], AF.Abs,
                            bias=sq_col[:, mi:mi + 1], scale=1.0,
                            accum_out=radacc[:, mi * 8 + c:mi * 8 + c + 1])

                rsum = p1w.tile([128, 1], F32, tag="rsum")
                nc.vector.tensor_reduce(rsum[:], radacc[:], axis=AX.X, op=OP.add)
                pt1 = ps1b.tile([128, 512], F32, tag="pt1")
                nc.tensor.matmul(pt1[0:1, 0:1], onc[:], rsum[:], start=True, stop=True)
                rad_sb = p1w.tile([1, 1], F32, tag="rad_sb")
                nc.vector.tensor_copy(rad_sb[:], pt1[0:1, 0:1])
                nc.sync.dma_start(d_rad_i[:], rad_sb[:])
                nc.gpsimd.collective_compute("AllGather", OP.bypass, replica_groups=rg,
                                             ins=[d_rad_i[:].opt()],
                                             outs=[d_rad_o[:].opt()])
                rad8 = p1w.tile([P, 1], F32, tag="rad8")
                nc.sync.dma_start(rad8[:], d_rad_o[:])
                pt2 = ps1b.tile([128, 512], F32, tag="pt1")
                nc.tensor.matmul(pt2[0:1, 0:1], onc[0:P, 0:1], rad8[:],
                                 start=True, stop=True)
                thr1 = p1w.tile([1, 1], F32, tag="thr1")
                nc.vector.tensor_scalar(out=thr1[:], in0=pt2[0:1, 0:1],
                                        scalar1=1.0 / (5.0 * float(N) * float(N)),
                                        scalar2=None, op0=OP.mult)
                pt3 = ps1b.tile([128, 512], F32, tag="pt1")
                nc.tensor.matmul(pt3[0:128, 0:1], onr[:], thr1[:], start=True, stop=True)
                thr_col = p1w.tile([128, 1], F32, tag="thr_col")
                nc.vector.tensor_copy(thr_col[:], pt3[0:128, 0:1])

                for mi in range(RC):
                    e8 = p1w.tile([128, N], FP8, tag="e8")
                    nc.vector.tensor_scalar(out=e8[:], in0=dist[:, mi, :],
                                            scalar1=thr_col[:], scalar2=None, op0=OP.is_lt)
                    nc.sync.dma_start(d_edge_i[mi * 128:(mi + 1) * 128, :], e8[:])
                nc.gpsimd.collective_compute("AllGather", OP.bypass, replica_groups=rg,
                                             ins=[d_edge_i[:].opt()],
                                             outs=[d_edge[:].opt()])

        # ---------- phase 2: H = edge_k @ edge > 0 ----------
        with tc.tile_pool(name="p2a", bufs=1) as p2a, \
             tc.tile_pool(name="p2w", bufs=3) as p2w, \
             tc.tile_pool(name="ps2", bufs=4, space="PSUM") as ps2:
            ef = p2a.tile([128, NC, N], FP8, tag="ef")
            ec = p2a.tile([128, NC, R], FP8, tag="ec")
            nc.sync.dma_start(ef[:], d_edge.rearrange("(jc p) j -> p jc j", p=128))
            nc.gpsimd.dma_start(
                ec[:],
                d_edge.rearrange("(jc p) j -> p jc j", p=128)[:, :, bass.ds(roffs, R)])
            degacc = p2a.tile([128, 32], F32, tag="degacc")
            for mi in range(RC):
                for c in range(NCC):
                    ph = ps2.tile([128, 512], F32, tag="ph")
                    for jp in range(NC // 2):
                        nc.tensor.matmul(
                            ph[:], ec[:, 2 * jp:2 * jp + 2, mi * 128:(mi + 1) * 128],
                            ef[:, 2 * jp:2 * jp + 2, c * 512:(c + 1) * 512],
                            start=(jp == 0), stop=(jp == NC // 2 - 1), perf_mode=DR)
                    hb = p2w.tile([128, 512], BF16, tag="hb")
                    nc.vector.tensor_scalar(out=hb[:], in0=ph[:], scalar1=0.0,
                                            scalar2=0.0, op0=OP.is_gt, op1=OP.add,
                                            accum_out=degacc[:, mi * 8 + c:mi * 8 + c + 1])
                    nc.sync.dma_start(
                        d_Hk[mi * 128:(mi + 1) * 128, c * 512:(c + 1) * 512], hb[:])
                    h8 = p2w.tile([128, 512], FP8, tag="h8")
                    nc.scalar.activation(h8[:], ph[:], AF.Sign)
                    nc.sync.dma_start(
                        d_H_i[mi * 128:(mi + 1) * 128, c * 512:(c + 1) * 512], h8[:])
                dg = p2w.tile([128, 1], F32, tag="dg")
                nc.vector.tensor_reduce(dg[:], degacc[:, mi * 8:(mi + 1) * 8], axis=AX.X,
                                        op=OP.add)
                rdg = p2w.tile([128, 1], F32, tag="rdg")
                nc.vector.reciprocal(rdg[:], dg[:])
                nc.scalar.activation(Dv_col[:, mi:mi + 1], rdg[:], AF.Sqrt)
            nc.sync.dma_start(d_Dv_i.rearrange("(b a) x -> a (b x)", b=RC), Dv_col[:])
            nc.gpsimd.collective_compute("AllGather", OP.bypass, replica_groups=rg,
                                         ins=[d_Dv_i[:].opt()], outs=[d_Dv[:].opt()])
            nc.gpsimd.collective_compute("AllGather", OP.bypass, replica_groups=rg,
                                         ins=[d_H_i[:].opt()], outs=[d_H[:].opt()])
            nc.sync.dma_start(Dv_all[:], d_Dv.rearrange("(jc p) x -> p (jc x)", p=128))
        Hc = pers.tile([128, NC, R], FP8, tag="Hc")
        nc.gpsimd.dma_start(
            Hc[:], d_H.rearrange("(jc p) j -> p jc j", p=128)[:, :, bass.ds(roffs, R)])

        # ---------- phase 3: V, U, E ----------
        with tc.tile_pool(name="p3a", bufs=1) as p3a, \
             tc.tile_pool(name="p3w", bufs=2) as p3w, \
             tc.tile_pool(name="ps3", bufs=2, space="PSUM") as ps3:
            V = p3a.tile([128, NC, F], BF16, tag="V")
            nc.sync.dma_start(V[:], Xn[:])
            for jc in range(NC):
                nc.vector.tensor_scalar(out=V[:, jc, :], in0=V[:, jc, :],
                                        scalar1=Dv_all[:, jc:jc + 1], scalar2=None,
                                        op0=OP.mult)
            Usc = p3a.tile([128, RC, F], BF16, tag="Usc")
            for mi in range(RC):
                pu = ps3.tile([128, 512], F32, tag="acc")
                for jc in range(NC):
                    nc.tensor.matmul(pu[:], Hc[:, jc, mi * 128:(mi + 1) * 128],
                                     V[:, jc, :], start=(jc == 0), stop=(jc == NC - 1))
                nc.vector.tensor_scalar(out=Usc[:, mi, :], in0=pu[:],
                                        scalar1=Dv_col[:, mi:mi + 1], scalar2=None,
                                        op0=OP.mult)
            UT = p3a.tile([128, FC, R], BF16, tag="UT")
            for mi in range(RC):
                for fc in range(FC):
                    pt = ps3.tile([128, 128], BF16, tag="pt")
                    nc.tensor.transpose(pt[:], Usc[:, mi, fc * 128:(fc + 1) * 128], idt[:])
                    nc.vector.tensor_copy(UT[:, fc, mi * 128:(mi + 1) * 128], pt[:])
            tht = p3a.tile([128, FC, HID], BF16, tag="tht")
            nc.sync.dma_start(tht[:], theta_t[:])
            for hc in range(HC):
                pe = ps3.tile([128, 512], F32, tag="acc")
                for fc in range(FC):
                    nc.tensor.matmul(pe[:], tht[:, fc, hc * 128:(hc + 1) * 128],
                                     UT[:, fc, :], start=(fc == 0), stop=(fc == FC - 1))
                nc.vector.tensor_copy(EkT[:, hc, :], pe[:])
            Enat = p3a.tile([128, RC, HID], BF16, tag="Enat")
            for hc in range(HC):
                for mi in range(RC):
                    pt = ps3.tile([128, 128], BF16, tag="pt")
                    nc.tensor.transpose(pt[:], EkT[:, hc, mi * 128:(mi + 1) * 128], idt[:])
                    nc.vector.tensor_copy(Enat[:, mi, hc * 128:(hc + 1) * 128], pt[:])
            nc.sync.dma_start(d_En_i.rearrange("(b a) h -> a b h", b=RC), Enat[:])
            nc.gpsimd.collective_compute("AllGather", OP.bypass, replica_groups=rg,
                                         ins=[d_En_i[:].opt()], outs=[d_En[:].opt()])

        # ---------- phase 4: Xh, then xx ----------
        pmid = _stk.enter_context(tc.tile_pool(name="pmid", bufs=1))
        with tc.tile_pool(name="p4a", bufs=1) as p4a, \
             tc.tile_pool(name="p4w", bufs=1) as p4w, \
             tc.tile_pool(name="p4s", bufs=3) as p4s, \
             tc.tile_pool(name="ps4", bufs=2, space="PSUM") as ps4:
            with tc.tile_pool(name="p4d", bufs=1) as p4d:
                DeE = p4d.tile([128, NC, HID], BF16, tag="DeE")
                nc.sync.dma_start(DeE[:], d_En.rearrange("(jc p) h -> p jc h", p=128))
                for jc in range(NC):
                    nc.vector.tensor_scalar(out=DeE[:, jc, :], in0=DeE[:, jc, :],
                                            scalar1=Dv_all[:, jc:jc + 1], scalar2=None,
                                            op0=OP.mult)
                Xhn = p4d.tile([128, RC, HID], BF16, tag="Xhn")
                for mi in range(RC):
                    px = ps4.tile([128, 512], F32, tag="acc")
                    for jc in range(NC):
                        nc.tensor.matmul(px[0:128, 0:HID],
                                         Hc[:, jc, mi * 128:(mi + 1) * 128],
                                         DeE[:, jc, :], start=(jc == 0),
                                         stop=(jc == NC - 1))
                    nc.vector.tensor_scalar(out=Xhn[:, mi, :], in0=px[0:128, 0:HID],
                                            scalar1=Dv_col[:, mi:mi + 1], scalar2=None,
                                            op0=OP.mult)
                for hc in range(HC):
                    for mi in range(RC):
                        pt = ps4.tile([128, 128], BF16, tag="pt")
                        nc.tensor.transpose(pt[:], Xhn[:, mi, hc * 128:(hc + 1) * 128],
                                            idt[:])
                        nc.vector.tensor_copy(XhkT[:, hc, mi * 128:(mi + 1) * 128], pt[:])
            nc.sync.dma_start(d_XhT_i.rearrange("(b a) h -> a b h", b=HC), XhkT[:])
            nc.gpsimd.collective_compute("AllGather", OP.bypass, replica_groups=rg,
                                         ins=[d_XhT_i[:].opt()], outs=[d_XhT[:].opt()])
            # batched v-vector locals: rows 0..4 from EkT (v1_h, v3), 5..8 from XhkT (v2_h)
            vE = p4w.tile([5, R], F32, tag="vE")
            pv9 = ps4.tile([128, 512], F32, tag="acc")
            for hc in range(HC):
                nc.tensor.matmul(pv9[0:5, :], wve_t[:, hc, 0:5], EkT[:, hc, :],
                                 start=(hc == 0), stop=(hc == HC - 1))
            nc.vector.tensor_copy(vE[:], pv9[0:5, :])
            vX = p4w.tile([4, R], F32, tag="vX")
            pv4 = ps4.tile([128, 512], F32, tag="acc")
            for hc in range(HC):
                nc.tensor.matmul(pv4[0:4, :], wvx_t[:, hc, 4:8], XhkT[:, hc, :],
                                 start=(hc == 0), stop=(hc == HC - 1))
            nc.vector.tensor_copy(vX[:], pv4[0:4, :])
            nc.sync.dma_start(d_v1_i[0:5, :], vE[:])
            nc.sync.dma_start(d_v1_i[5:9, :], vX[:])
            nc.gpsimd.collective_compute("AllGather", OP.bypass, replica_groups=rg,
                                         ins=[d_v1_i[:].opt()], outs=[d_v1[:].opt()])

            xx = pmid.tile([128, RC, N], FP8, tag="xx")
            Hf = p4a.tile([128, NC, N], FP8, tag="Hf")
            nc.sync.dma_start(Hf[:], d_H.rearrange("(jc p) j -> p jc j", p=128))
            for mi in range(RC):
                sel = p4w.tile([128, N], BF16, tag="sel")
                nc.sync.dma_start(sel[:], selbig[:, mi, :])
                for c in range(NCC):
                    pxx = ps4.tile([128, 512], F32, tag="acc2")
                    for jp in range(NC // 2):
                        nc.tensor.matmul(
                            pxx[:], Hc[:, 2 * jp:2 * jp + 2, mi * 128:(mi + 1) * 128],
                            Hf[:, 2 * jp:2 * jp + 2, c * 512:(c + 1) * 512],
                            start=(jp == 0), stop=(jp == NC // 2 - 1), perf_mode=DR)
                    xv = p4s.tile([128, 512], F32, tag="xv")
                    nc.vector.scalar_tensor_tensor(
                        out=xv[:], in0=pxx[:], scalar=1.0,
                        in1=sel[:, c * 512:(c + 1) * 512], op0=OP.mult, op1=OP.add)
                    nc.vector.tensor_scalar(out=xx[:, mi, c * 512:(c + 1) * 512],
                                            in0=xv[:], scalar1=0.0, scalar2=None,
                                            op0=OP.is_gt)


        xx = xx  # noqa: F821  (assigned in phase 4 scope above)

        # ---------- shared helpers ----------
        def inv_norm_row(mT, dram_row, invc_out, pool, psw, tag):
            """mT [128, HC, N] -> inverse column norms [1, N] f32 (returned tile)
            and the local per-partition slices invc_out [128, RC]."""
            s2 = pool.tile([128, HC, N], F32, tag=tag + "s2", bufs=1)
            for hc in range(HC):
                nc.vector.scalar_tensor_tensor(out=s2[:, hc, :], in0=mT[:, hc, :],
                                               scalar=1.0, in1=mT[:, hc, :],
                                               op0=OP.mult, op1=OP.mult)
            n2r = pool.tile([1, N], F32, tag=tag + "r", bufs=1)
            for c in range(NCC):
                pr = psw.tile([128, 512], F32, tag="acc")
                for hc in range(HC):
                    nc.tensor.matmul(pr[0:1, :], onc[:], s2[:, hc, c * 512:(c + 1) * 512],
                                     start=(hc == 0), stop=(hc == HC - 1))
                nc.vector.tensor_copy(n2r[0:1, c * 512:(c + 1) * 512], pr[0:1, :])
            rn = pool.tile([1, N], F32, tag=tag + "rn", bufs=1)
            nc.vector.reciprocal(rn[:], n2r[:])
            nc.scalar.activation(n2r[:], rn[:], AF.Sqrt)
            nc.sync.dma_start(dram_row[:], n2r[:])
            nc.gpsimd.dma_start(
                invc_out,
                dram_row[0:1, bass.ds(roffs, R)].rearrange("x (a b) -> x b a", a=RC))
            return n2r

        def rho_pass(kT, fT, invc, invr, rho_dram_i, rho_dram_o, pool, psw, tag):
            invb = pool.tile([128, N], BF16, tag=tag + "invb", bufs=1)
            for c in range(NCC):
                pb = psw.tile([128, 512], F32, tag="bc")
                nc.tensor.matmul(pb[:], onr[:], invr[0:1, c * 512:(c + 1) * 512],
                                 start=True, stop=True)
                nc.vector.tensor_copy(invb[:, c * 512:(c + 1) * 512], pb[:])
            racc = pool.tile([128, 32], F32, tag=tag + "acc")
            for mi in range(RC):
                for c in range(NCC):
                    pC = psw.tile([128, 512], F32, tag="acc")
                    for hc in range(HC):
                        nc.tensor.matmul(pC[:], kT[:, hc, mi * 128:(mi + 1) * 128],
                                         fT[:, hc, c * 512:(c + 1) * 512],
                                         start=(hc == 0), stop=(hc == HC - 1))
                    q = pool.tile([128, 512], BF16, tag=tag + "q")
                    nc.vector.scalar_tensor_tensor(out=q[:], in0=pC[:],
                                                   scalar=invc[:, mi:mi + 1],
                                                   in1=invb[:, c * 512:(c + 1) * 512],
                                                   op0=OP.mult, op1=OP.mult)
                    t = pool.tile([128, 512], BF16, tag=tag + "t")
                    nc.vector.tensor_scalar(out=t[:], in0=q[:], scalar1=SIGMA,
                                            scalar2=None, op0=OP.is_gt)
                    xc = pool.tile([128, 512], BF16, tag=tag + "xc")
                    nc.vector.tensor_copy(xc[:], xx[:, mi, c * 512:(c + 1) * 512])
                    tx = pool.tile([128, 512], BF16, tag=tag + "tx")
                    nc.vector.tensor_tensor(tx[:], t[:], xc[:], op=OP.mult)
                    nc.vector.scalar_tensor_tensor(
                        out=t[:], in0=q[:], scalar=1.0, in1=tx[:], op0=OP.mult,
                        op1=OP.mult, accum_out=racc[:, mi * 8 + c:mi * 8 + c + 1])
            rloc = pool.tile([128, RC], F32, tag=tag + "rl")
            for mi in range(RC):
                nc.vector.tensor_reduce(rloc[:, mi:mi + 1], racc[:, mi * 8:(mi + 1) * 8],
                                        axis=AX.X, op=OP.add)
            nc.sync.dma_start(rho_dram_i.rearrange("(b a) x -> a (b x)", b=RC), rloc[:])
            nc.gpsimd.collective_compute("AllGather", OP.bypass, replica_groups=rg,
                                         ins=[rho_dram_i[:].opt()],
                                         outs=[rho_dram_o[:].opt()])

        def rho_e_pass(rho_dram, re_dram_i, re_dram_o, pool, psw, tag):
            rb32 = pool.tile([128, NC], F32, tag=tag + "b32")
            nc.sync.dma_start(rb32[:], rho_dram.rearrange("(jc p) x -> p (jc x)", p=128))
            rbf = pool.tile([128, NC], BF16, tag=tag + "bf")
            nc.vector.tensor_copy(rbf[:], rb32[:])
            rec = pool.tile([128, RC], F32, tag=tag + "rec")
            for mi in range(RC):
                pre = psw.tile([128, 512], F32, tag="acc")
                for jc in range(NC):
                    nc.tensor.matmul(pre[0:128, 0:1], Hc[:, jc, mi * 128:(mi + 1) * 128],
                                     rbf[:, jc:jc + 1], start=(jc == 0),
                                     stop=(jc == NC - 1))
                nc.vector.tensor_copy(rec[:, mi:mi + 1], pre[0:128, 0:1])
            nc.sync.dma_start(re_dram_i.rearrange("(b a) x -> a (b x)", b=RC), rec[:])
            nc.gpsimd.collective_compute("AllGather", OP.bypass, replica_groups=rg,
                                         ins=[re_dram_i[:].opt()],
                                         outs=[re_dram_o[:].opt()])

        def flag_scale_from(rho_dram, pool, tag):
            mx8 = pool.tile([1, NCC], F32, tag=tag + "mx8")
            mn8 = pool.tile([1, NCC], F32, tag=tag + "mn8")
            for c in range(NCC):
                rr = pool.tile([1, 512], F32, tag=tag + "row")
                nc.sync.dma_start(rr[:],
                                  rho_dram.rearrange("a x -> x a")[0:1,
                                                                  c * 512:(c + 1) * 512])
                nc.vector.tensor_reduce(mx8[0:1, c:c + 1], rr[:], axis=AX.X, op=OP.max)
                nc.vector.tensor_reduce(mn8[0:1, c:c + 1], rr[:], axis=AX.X, op=OP.min)
            mx = pool.tile([1, 1], F32, tag=tag + "mx")
            mn = pool.tile([1, 1], F32, tag=tag + "mn")
            nc.vector.tensor_reduce(mx[:], mx8[:], axis=AX.X, op=OP.max)
            nc.vector.tensor_reduce(mn[:], mn8[:], axis=AX.X, op=OP.min)
            fl = pool.tile([1, 1], F32, tag=tag + "fl")
            nc.vector.tensor_tensor(fl[:], mx[:], mn[:], op=OP.is_equal)
            nc.vector.tensor_scalar(out=fl[:], in0=fl[:], scalar1=-1.0, scalar2=1.0,
                                    op0=OP.mult, op1=OP.add)
            return fl

        def scale_after_elu(dst, src_dram, flag_dram, nparts, pool, psw, tag,
                            dst_dram=None, dst_dtype=None):
            """dst[nparts, N] = elu(AR result) * (1 - degenerate_flag).
            If dst_dram given, stream chunks there instead (dst ignored)."""
            fl = flag_scale_from(flag_dram, pool, tag + "f")
            pf = psw.tile([128, 512], F32, tag="bc")
            nc.tensor.matmul(pf[0:nparts, 0:1], onr[0:1, 0:nparts], fl[:],
                             start=True, stop=True)
            fc = pool.tile([nparts, 1], F32, tag=tag + "fc")
            nc.vector.tensor_copy(fc[:], pf[0:nparts, 0:1])
            for c in range(NCC):
                cs = slice(c * 512, (c + 1) * 512)
                ar = pool.tile([nparts, 512], F32, tag=tag + "ar")
                nc.sync.dma_start(ar[:], src_dram[:, cs])
                r_ = pool.tile([nparts, 512], F32, tag=tag + "r")
                nc.scalar.activation(r_[:], ar[:], AF.Relu)
                zmr = pool.tile([nparts, 512], F32, tag=tag + "z")
                nc.vector.scalar_tensor_tensor(out=zmr[:], in0=ar[:], scalar=1.0,
                                               in1=r_[:], op0=OP.mult, op1=OP.subtract)
                ex = pool.tile([nparts, 512], F32, tag=tag + "e")
                nc.scalar.activation(ex[:], zmr[:], AF.Exp)
                el = pool.tile([nparts, 512], F32, tag=tag + "el")
                nc.vector.scalar_tensor_tensor(out=el[:], in0=ex[:], scalar=-1.0,
                                               in1=r_[:], op0=OP.add, op1=OP.add)
                if dst_dram is None:
                    nc.vector.tensor_scalar(out=dst[:, cs], in0=el[:], scalar1=fc[:],
                                            scalar2=None, op0=OP.mult)
                else:
                    oc = pool.tile([nparts, 512], dst_dtype, tag=tag + "oc")
                    nc.vector.tensor_scalar(out=oc[:], in0=el[:], scalar1=fc[:],
                                            scalar2=None, op0=OP.mult)
                    nc.sync.dma_start(dst_dram[:, cs], oc[:])

        def fetch_vrow(v_dram, nv, idx, pool, tag):
            vr = pool.tile([1, N], F32, tag="vrow", name=tag, bufs=1)
            for c in range(P):
                nc.sync.dma_start(vr[0:1, c * R:(c + 1) * R],
                                  v_dram[c * nv + idx:c * nv + idx + 1, :])
            return vr

        def ucol_mm(wtile, col, locT, pool, psw, tag):
            uc = pool.tile([128, RC], F32, tag=tag)
            for mi in range(RC):
                pu = psw.tile([128, 512], F32, tag="acc")
                for hc in range(HC):
                    nc.tensor.matmul(pu[0:128, 0:1], locT[:, hc, mi * 128:(mi + 1) * 128],
                                     wtile[:, hc, col:col + 1],
                                     start=(hc == 0), stop=(hc == HC - 1))
                nc.vector.tensor_copy(uc[:, mi:mi + 1], pu[0:128, 0:1])
            return uc

        plate = _stk.enter_context(tc.tile_pool(name="plate", bufs=1))
        Hkb = plate.tile([128, RC, N], BF16, tag="Hkb")
        nc.sync.dma_start(Hkb[:], d_Hk.rearrange("(b a) j -> a b j", b=RC))

        def attention(u_col, v_row, agg, agg_off, dst_dram, dst_row0, pool, psw, tag):
            """Masked softmax attention for local rows; DMAs the partial
            (att^T @ agg') into dst_dram[dst_row0:dst_row0+8, :] (f32)."""
            vmax = pool.tile([1, 1], F32, tag=tag + "vm")
            nc.vector.tensor_reduce(vmax[:], v_row[:], axis=AX.X, op=OP.max)
            pvb = psw.tile([128, 512], F32, tag="bc")
            nc.tensor.matmul(pvb[0:128, 0:1], onr[:], vmax[:], start=True, stop=True)
            vmc = pool.tile([128, 1], F32, tag=tag + "vmc")
            nc.vector.tensor_copy(vmc[:], pvb[0:128, 0:1])
            nb = pool.tile([128, RC], F32, tag=tag + "nb")
            for mi in range(RC):
                nc.scalar.activation(nb[:, mi:mi + 1], u_col[:, mi:mi + 1], AF.Prelu,
                                     bias=vmc[:], scale=1.0, alpha=0.2)
            nc.vector.tensor_scalar(out=nb[:], in0=nb[:], scalar1=-1.0, scalar2=None,
                                    op0=OP.mult)
            Vb = pool.tile([128, N], BF16, tag=tag + "Vb", bufs=1)
            for c in range(NCC):
                pbb = psw.tile([128, 512], F32, tag="bc")
                nc.tensor.matmul(pbb[:], onr[:], v_row[0:1, c * 512:(c + 1) * 512],
                                 start=True, stop=True)
                nc.vector.tensor_copy(Vb[:, c * 512:(c + 1) * 512], pbb[:])
            em = pool.tile([128, RC, N], BF16, tag=tag + "em", bufs=1)
            ag = pool.tile([128, RC, 8], BF16, tag=tag + "ag")
            racc = pool.tile([128, 8], F32, tag=tag + "racc")
            for mi in range(RC):
                for c in range(NCC):
                    s = pool.tile([128, 512], BF16, tag=tag + "s")
                    nc.scalar.activation(s[:], Vb[:, c * 512:(c + 1) * 512], AF.Prelu,
                                         bias=u_col[:, mi:mi + 1], scale=1.0, alpha=0.2)
                    e0 = pool.tile([128, 512], BF16, tag=tag + "e0")
                    nc.scalar.activation(e0[:], s[:], AF.Exp, bias=nb[:, mi:mi + 1],
                                         scale=1.0)
                    nc.vector.scalar_tensor_tensor(
                        out=em[:, mi, c * 512:(c + 1) * 512], in0=e0[:], scalar=1.0,
                        in1=Hkb[:, mi, c * 512:(c + 1) * 512], op0=OP.mult, op1=OP.mult,
                        accum_out=racc[:, c:c + 1])
                rc_ = pool.tile([128, 1], F32, tag=tag + "rc")
                nc.vector.tensor_reduce(rc_[:], racc[:], axis=AX.X, op=OP.add)
                ir = pool.tile([128, 1], F32, tag=tag + "ir")
                nc.vector.reciprocal(ir[:], rc_[:])
                nc.vector.tensor_scalar(out=ag[:, mi, :],
                                        in0=agg[:, mi, agg_off:agg_off + 8],
                                        scalar1=ir[:], scalar2=None, op0=OP.mult)
            for c in range(NCC):
                pat = psw.tile([8, 512], F32, tag="pat")
                for mi in range(RC):
                    nc.tensor.matmul(pat[:], ag[:, mi, :],
                                     em[:, mi, c * 512:(c + 1) * 512],
                                     start=(mi == 0), stop=(mi == RC - 1))
                pc_sb = pool.tile([8, 512], F32, tag=tag + "pc")
                nc.scalar.copy(pc_sb[:], pat[:])
                nc.sync.dma_start(
                    dst_dram[dst_row0:dst_row0 + 8, c * 512:(c + 1) * 512], pc_sb[:])

        # ---------- phase 5: rho1 / rho_e1 ----------
        with tc.tile_pool(name="p5", bufs=2) as p5, \
             tc.tile_pool(name="p5b", bufs=1) as p5b, \
             tc.tile_pool(name="ps5", bufs=2, space="PSUM") as ps5:
            XhT = p5b.tile([128, HC, N], BF16, tag="XhT")
            for c in range(P):
                for hc in range(HC):
                    nc.sync.dma_start(
                        XhT[:, hc, c * R:(c + 1) * R],
                        d_XhT[c * HID + hc * 128:c * HID + (hc + 1) * 128, :])
            invc1 = plate.tile([128, RC], F32, tag="invc1")
            invr1 = inv_norm_row(XhT, d_in1, invc1[:], p5b, ps5, "in1")
            rho_pass(XhkT, XhT, invc1, invr1, d_rho1_i, d_rho1, p5, ps5, "r1")
            rho_e_pass(d_rho1, d_re1_i, d_re1, p5, ps5, "re1")

        # ---------- phase 6: attention-1 for 4 heads -> EnewT ----------
        with tc.tile_pool(name="p6", bufs=2) as p6, \
             tc.tile_pool(name="p6p", bufs=1) as p6p, \
             tc.tile_pool(name="ps6", bufs=2, space="PSUM") as ps6:
            WXk = p6p.tile([128, RC, 32], BF16, tag="WXk")
            for h in range(4):
                for mi in range(RC):
                    pw = ps6.tile([128, 512], F32, tag="acc")
                    for hc in range(HC):
                        nc.tensor.matmul(pw[0:128, 0:8],
                                         XhkT[:, hc, mi * 128:(mi + 1) * 128],
                                         wcat_t[:, hc, h * 8:(h + 1) * 8],
                                         start=(hc == 0), stop=(hc == HC - 1))
                    nc.vector.tensor_copy(WXk[:, mi, h * 8:(h + 1) * 8], pw[0:128, 0:8])
            for h in range(4):
                u1 = ucol_mm(wvx_t, h, XhkT, p6, ps6, "u1")
                v1 = fetch_vrow(d_v1, 9, h, p6, "v1")
                attention(u1, v1, WXk, h * 8, d_p1_i, h * 8, p6, ps6, "a1")
            nc.gpsimd.collective_compute("AllReduce", OP.add, replica_groups=rg,
                                         ins=[d_p1_i[:].opt()], outs=[d_p1[:].opt()])
            scale_after_elu(None, d_p1, d_rho1, 32, p6p, ps6, "s1",
                            dst_dram=d_EnT, dst_dtype=BF16)

        E2n = plate.tile([128, RC, 32], BF16, tag="E2n")
        with tc.tile_pool(name="p6b", bufs=2) as p6b, \
             tc.tile_pool(name="ps6b", bufs=2, space="PSUM") as ps6b:
            enl = p6b.tile([32, R], BF16, tag="enl")
            nc.gpsimd.dma_start(enl[:], d_EnT[:, bass.ds(roffs, R)])
            for mi in range(RC):
                pt = ps6b.tile([128, 128], BF16, tag="pt")
                nc.tensor.transpose(pt[0:128, 0:32], enl[:, mi * 128:(mi + 1) * 128],
                                    idt[0:32, 0:32])
                nc.vector.tensor_copy(E2n[:, mi, :], pt[0:128, 0:32])

        # ---------- phase 7: attention-2 -> XcT -> XlT ----------
        with tc.tile_pool(name="p7", bufs=2) as p7, \
             tc.tile_pool(name="p7p", bufs=1) as p7p, \
             tc.tile_pool(name="ps7", bufs=2, space="PSUM") as ps7:
            for h in range(4):
                u2 = ucol_mm(wve_t, 5 + h, EkT, p7, ps7, "u2")
                v2 = fetch_vrow(d_v1, 9, 5 + h, p7, "v2")
                attention(u2, v2, E2n, h * 8, d_p2_i, h * 8, p7, ps7, "a2")
            nc.gpsimd.collective_compute("AllReduce", OP.add, replica_groups=rg,
                                         ins=[d_p2_i[:].opt()], outs=[d_p2[:].opt()])
            XcT = p7p.tile([32, N], BF16, tag="XcT")
            scale_after_elu(XcT[:], d_p2, d_re1, 32, p7p, ps7, "s2")

            lw = p7p.tile([32, HID], BF16, tag="lw")
            nc.sync.dma_start(lw[:], linw[:])
            lb = p7p.tile([128, HC], F32, tag="lb")
            nc.sync.dma_start(lb[:], linb[:])
            for hc in range(HC):
                for c in range(NCC):
                    pxl = ps7.tile([128, 512], F32, tag="acc")
                    nc.tensor.matmul(pxl[:], lw[:, hc * 128:(hc + 1) * 128],
                                     XcT[:, c * 512:(c + 1) * 512], start=True, stop=True)
                    r_ = p7.tile([128, 512], F32, tag="xlr")
                    nc.scalar.activation(r_[:], pxl[:], AF.Relu, bias=lb[:, hc:hc + 1],
                                         scale=1.0)
                    zmr = p7.tile([128, 512], F32, tag="xlz")
                    nc.vector.scalar_tensor_tensor(out=zmr[:], in0=pxl[:],
                                                   scalar=lb[:, hc:hc + 1], in1=r_[:],
                                                   op0=OP.add, op1=OP.subtract)
                    ex = p7.tile([128, 512], F32, tag="xle")
                    nc.scalar.activation(ex[:], zmr[:], AF.Exp)
                    xlo = p7.tile([128, 512], BF16, tag="xlo")
                    nc.vector.scalar_tensor_tensor(out=xlo[:], in0=ex[:], scalar=-1.0,
                                                   in1=r_[:], op0=OP.add, op1=OP.add)
                    nc.sync.dma_start(
                        d_XlT.rearrange("(b p) j -> p b j", p=128)[:, hc,
                                                                  c * 512:(c + 1) * 512],
                        xlo[:])

        XlkT = plate.tile([128, HC, R], BF16, tag="XlkT")
        nc.gpsimd.dma_start(
            XlkT[:],
            d_XlT.rearrange("(b p) j -> p b j", p=128)[:, :, bass.ds(roffs, R)])

        # ---------- phase 8: rho2 / rho_e2 + v4 ----------
        with tc.tile_pool(name="p8", bufs=2) as p8, \
             tc.tile_pool(name="p8b", bufs=1) as p8b, \
             tc.tile_pool(name="ps8", bufs=2, space="PSUM") as ps8:
            XlT = p8b.tile([128, HC, N], BF16, tag="XlT")
            nc.sync.dma_start(XlT[:], d_XlT.rearrange("(b p) j -> p b j", p=128))
            vl = p8.tile([1, R], F32, tag="vl")
            pvl = ps8.tile([128, 512], F32, tag="acc")
            for hc in range(HC):
                nc.tensor.matmul(pvl[0:1, :], wvl_t[:, hc, 1:2], XlkT[:, hc, :],
                                 start=(hc == 0), stop=(hc == HC - 1))
            nc.vector.tensor_copy(vl[:], pvl[0:1, :])
            nc.sync.dma_start(d_v2_i[:], vl[:])
            nc.gpsimd.collective_compute("AllGather", OP.bypass, replica_groups=rg,
                                         ins=[d_v2_i[:].opt()], outs=[d_v2[:].opt()])
            invc2 = plate.tile([128, RC], F32, tag="invc2")
            invr2 = inv_norm_row(XlT, d_in2, invc2[:], p8b, ps8, "in2")
            rho_pass(XlkT, XlT, invc2, invr2, d_rho2_i, d_rho2, p8, ps8, "r2")
            rho_e_pass(d_rho2, d_re2_i, d_re2, p8, ps8, "re2")

        # ---------- phase 9: final attentions ----------
        with tc.tile_pool(name="p9", bufs=2) as p9, \
             tc.tile_pool(name="p9p", bufs=1) as p9p, \
             tc.tile_pool(name="ps9", bufs=2, space="PSUM") as ps9:
            WX2 = p9p.tile([128, RC, 8], BF16, tag="WX2")
            for mi in range(RC):
                pw = ps9.tile([128, 512], F32, tag="acc")
                for hc in range(HC):
                    nc.tensor.matmul(pw[0:128, 0:8], XlkT[:, hc, mi * 128:(mi + 1) * 128],
                                     wcat_t[:, hc, 32:40],
                                     start=(hc == 0), stop=(hc == HC - 1))
                nc.vector.tensor_copy(WX2[:, mi, :], pw[0:128, 0:8])
            u3 = ucol_mm(wvl_t, 0, XlkT, p9, ps9, "u3")
            v3 = fetch_vrow(d_v1, 9, 4, p9, "v3")
            attention(u3, v3, WX2, 0, d_p3_i, 0, p9, ps9, "a3")
            nc.gpsimd.collective_compute("AllReduce", OP.add, replica_groups=rg,
                                         ins=[d_p3_i[:].opt()], outs=[d_p3[:].opt()])
            scale_after_elu(None, d_p3, d_rho2, 8, p9p, ps9, "s3",
                            dst_dram=d_E2T, dst_dtype=BF16)
            e2l = p9.tile([8, R], BF16, tag="e2l")
            nc.gpsimd.dma_start(e2l[:], d_E2T[:, bass.ds(roffs, R)])
            E3n = p9p.tile([128, RC, 8], BF16, tag="E3n")
            for mi in range(RC):
                pt = ps9.tile([128, 128], BF16, tag="pt")
                nc.tensor.transpose(pt[0:128, 0:8], e2l[:, mi * 128:(mi + 1) * 128],
                                    idt[0:8, 0:8])
                nc.vector.tensor_copy(E3n[:, mi, :], pt[0:128, 0:8])
            u4 = ucol_mm(wve_t, 9, EkT, p9, ps9, "u4")
            v4 = fetch_vrow(d_v2, 1, 0, p9, "v4")
            attention(u4, v4, E3n, 0, d_p4_i, 0, p9, ps9, "a4")
            nc.gpsimd.collective_compute("AllReduce", OP.add, replica_groups=rg,
                                         ins=[d_p4_i[:].opt()], outs=[d_p4[:].opt()])
            scale_after_elu(None, d_p4, d_re2, 8, p9p, ps9, "s3",
                            dst_dram=out_ext, dst_dtype=F32)
    nc.finalize()
    return nc


_BUILT = None


def _prep_inputs(X, theta, W_heads, ax_heads, ae_heads, lin_w, lin_b, W2, ax2, ae2):
    bf = ml_dtypes.bfloat16
    X = np.asarray(X, np.float32)
    Xb = X.astype(bf)
    XTh = np.ascontiguousarray(Xb.T)                             # [F, N]
    XT_t = XTh.reshape(FC, 128, N).transpose(1, 0, 2).copy()     # [128, FC, N]
    Xn_t = Xb.reshape(NC, 128, F).transpose(1, 0, 2).copy()      # [128, NC, F]
    th_t = (np.asarray(theta, np.float32).astype(bf)
            .reshape(FC, 128, HID).transpose(1, 0, 2).copy())
    Wl = [np.asarray(W_heads[i], np.float32) for i in range(4)] + \
         [np.asarray(W2, np.float32)]
    Wcat_f = np.concatenate(Wl, axis=1)                          # [HID, 40]
    Wcat_t = Wcat_f.astype(bf).reshape(HC, 128, 40).transpose(1, 0, 2).copy()

    axl = [np.asarray(a, np.float32) for a in ax_heads]
    ael = [np.asarray(a, np.float32) for a in ae_heads]
    ax2 = np.asarray(ax2, np.float32)
    ae2 = np.asarray(ae2, np.float32)
    # wvx (Xh-side): cols 0..3 u of att1 (W@ax[:D]);  4..7 v of att2 (W@ae[D:])
    wvx_cols = [Wl[i] @ axl[i][:D] for i in range(4)] + \
               [Wl[i] @ ael[i][D:] for i in range(4)]
    # wve (E-side): cols 0..3 v of att1 (W@ax[D:]); col 4 v3 (W2@ax2[D:]);
    #               cols 5..8 u of att2 (W@ae[:D]); col 9 u4 (W2@ae2[:D])
    wve_cols = [Wl[i] @ axl[i][D:] for i in range(4)] + \
               [Wl[4] @ ax2[D:]] + \
               [Wl[i] @ ael[i][:D] for i in range(4)] + \
               [Wl[4] @ ae2[:D]]
    # wvl (Xl-side): col 0 u3 (W2@ax2[:D]); col 1 v4 (W2@ae2[D:])
    wvl_cols = [Wl[4] @ ax2[:D], Wl[4] @ ae2[D:]]
    wvx_f = np.concatenate(wvx_cols, axis=1)
    wve_f = np.concatenate(wve_cols, axis=1)
    wvl_f = np.concatenate(wvl_cols, axis=1)
    wvx_t = wvx_f.astype(bf).reshape(HC, 128, 8).transpose(1, 0, 2).copy()
    wve_t = wve_f.astype(bf).reshape(HC, 128, 10).transpose(1, 0, 2).copy()
    wvl_t = wvl_f.astype(bf).reshape(HC, 128, 2).transpose(1, 0, 2).copy()
    linw_t = np.asarray(lin_w, np.float32).astype(bf).copy()
    linb_t = np.asarray(lin_b, np.float32).reshape(HC, 128).T.copy()
    ident = np.eye(128, dtype=bf)

    common = dict(XT=XT_t, Xn=Xn_t, theta_t=th_t, Wcat=Wcat_t, wvx=wvx_t, wve=wve_t,
                  wvl=wvl_t, linw=linw_t, linb=linb_t,
                  ones_row=np.ones((1, 128), np.float32),
                  ones_col=np.ones((128, 1), np.float32), ident=ident)
    maps = []
    pr = np.arange(128)
    for k in range(P):
        sel = np.zeros((128, RC, N), bf)
        for mi in range(RC):
            sel[pr, mi, k * R + mi * 128 + pr] = bf(-1e9)
        m = dict(common)
        m["XTk"] = XT_t[:, :, k * R:(k + 1) * R].copy()
        m["selbig"] = sel
        m["roff_in"] = np.array([[k * R]], np.uint32)
        maps.append(m)
    return maps


def kernel(**inputs):
    global _BUILT
    if _BUILT is None:
        _BUILT = build()
    maps = _prep_inputs(**inputs)
    res = run_bass_kernel_spmd(_BUILT, maps, core_ids=list(range(P)))
    outT = res.results[0]["out"]
    return np.ascontiguousarray(np.asarray(outT).T).astype(np.float32)


# revision 21
# speedup vs baseline: 1.1554x; 1.1554x over previous
"""DA-HGNN TRN2 Bass kernel: 8-core row-sharded SPMD implementation.

Self-contained: hardcodes shapes n=4096, F=512, hid=256, d=8, heads=4.
kernel(**inputs) takes full unsharded inputs, returns full (4096, 8) f32.

Math notes vs reference.py:
- edge/H/xx products use fp8 (operands are exactly 0/1; fp32 PSUM accumulate
  => bit-exact counts), with DoubleRow perf mode.
- softmax is shift-invariant per row, so the rho_t shift has no effect on the
  output when rho is non-constant.  When rho is constant the reference (on the
  neuron backend) yields elu(NaN)=0 rows; we reproduce that exactly with a
  multiplicative flag (1 - [max(rho)==min(rho)]) applied after elu.  No NaNs
  ever materialize on the device.
"""
from contextlib import ExitStack
import numpy as np
import ml_dtypes

import concourse.bass as bass
import concourse.bacc as bacc
import concourse.mybir as mybir
import concourse.tile as tile
from concourse.bass_utils import run_bass_kernel_spmd

F32 = mybir.dt.float32
BF16 = mybir.dt.bfloat16
FP8 = mybir.dt.float8e4
U32 = mybir.dt.uint32
AF = mybir.ActivationFunctionType
OP = mybir.AluOpType
AX = mybir.AxisListType
DR = mybir.MatmulPerfMode.DoubleRow

P = 8
N = 4096
F = 512
HID = 256
D = 8
R = N // P        # 512
RC = R // 128     # 4
NC = N // 128     # 32
FC = F // 128     # 4
HC = HID // 128   # 2
NCC = N // 512    # 8
SIGMA = 0.5


def build():
    nc = bacc.Bacc(None, num_devices=P)

    XT = nc.declare_dram_parameter("XT", [128, FC, N], BF16, isOutput=False)
    XTk = nc.declare_dram_parameter("XTk", [128, FC, R], BF16, isOutput=False)
    Xn = nc.declare_dram_parameter("Xn", [128, NC, F], BF16, isOutput=False)
    theta_t = nc.declare_dram_parameter("theta_t", [128, FC, HID], BF16, isOutput=False)
    Wcat = nc.declare_dram_parameter("Wcat", [128, HC, 40], BF16, isOutput=False)
    wvx = nc.declare_dram_parameter("wvx", [128, HC, 8], BF16, isOutput=False)
    wve = nc.declare_dram_parameter("wve", [128, HC, 10], BF16, isOutput=False)
    wvl = nc.declare_dram_parameter("wvl", [128, HC, 2], BF16, isOutput=False)
    linw = nc.declare_dram_parameter("linw", [32, HID], BF16, isOutput=False)
    linb = nc.declare_dram_parameter("linb", [128, HC], F32, isOutput=False)
    ones_row = nc.declare_dram_parameter("ones_row", [1, 128], F32, isOutput=False)
    ones_col = nc.declare_dram_parameter("ones_col", [128, 1], F32, isOutput=False)
    ident = nc.declare_dram_parameter("ident", [128, 128], BF16, isOutput=False)
    selbig = nc.declare_dram_parameter("selbig", [128, RC, N], BF16, isOutput=False)
    roff_in = nc.declare_dram_parameter("roff_in", [1, 1], U32, isOutput=False)

    out_ext = nc.declare_dram_parameter("out", [8, N], F32, isOutput=True)

    rg = [list(range(P))]
    d_sq = nc.dram_tensor("d_sq", [1, N], F32)
    d_rad_i = nc.dram_tensor("d_rad_i", [1, 1], F32)
    d_rad_o = nc.dram_tensor("d_rad_o", [P, 1], F32, addr_space="Shared")
    d_edge_i = nc.dram_tensor("d_edge_i", [R, N], FP8)
    d_edge = nc.dram_tensor("d_edge", [N, N], FP8, addr_space="Shared")
    d_H_i = nc.dram_tensor("d_H_i", [R, N], FP8)
    d_H = nc.dram_tensor("d_H", [N, N], FP8, addr_space="Shared")
    d_Hk = nc.dram_tensor("d_Hk", [R, N], BF16)
    d_Dv_i = nc.dram_tensor("d_Dv_i", [R, 1], F32)
    d_Dv = nc.dram_tensor("d_Dv", [N, 1], F32, addr_space="Shared")
    d_En_i = nc.dram_tensor("d_En_i", [R, HID], BF16)
    d_En = nc.dram_tensor("d_En", [N, HID], BF16, addr_space="Shared")
    d_v1_i = nc.dram_tensor("d_v1_i", [9, R], F32)
    d_v1 = nc.dram_tensor("d_v1", [9 * P, R], F32, addr_space="Shared")
    d_v2_i = nc.dram_tensor("d_v2_i", [1, R], F32)
    d_v2 = nc.dram_tensor("d_v2", [P, R], F32, addr_space="Shared")
    d_XhT_i = nc.dram_tensor("d_XhT_i", [HID, R], BF16)
    d_XhT = nc.dram_tensor("d_XhT", [HID * P, R], BF16, addr_space="Shared")
    d_rho1_i = nc.dram_tensor("d_rho1_i", [R, 1], F32)
    d_rho1 = nc.dram_tensor("d_rho1", [N, 1], F32, addr_space="Shared")
    d_re1_i = nc.dram_tensor("d_re1_i", [R, 1], F32)
    d_re1 = nc.dram_tensor("d_re1", [N, 1], F32, addr_space="Shared")
    d_rho2_i = nc.dram_tensor("d_rho2_i", [R, 1], F32)
    d_rho2 = nc.dram_tensor("d_rho2", [N, 1], F32, addr_space="Shared")
    d_re2_i = nc.dram_tensor("d_re2_i", [R, 1], F32)
    d_re2 = nc.dram_tensor("d_re2", [N, 1], F32, addr_space="Shared")
    d_p1_i = nc.dram_tensor("d_p1_i", [32, N], F32)
    d_p1 = nc.dram_tensor("d_p1", [32, N], F32, addr_space="Shared")
    d_p2_i = nc.dram_tensor("d_p2_i", [32, N], F32)
    d_p2 = nc.dram_tensor("d_p2", [32, N], F32, addr_space="Shared")
    d_p3_i = nc.dram_tensor("d_p3_i", [8, N], F32)
    d_p3 = nc.dram_tensor("d_p3", [8, N], F32, addr_space="Shared")
    d_p4_i = nc.dram_tensor("d_p4_i", [8, N], F32)
    d_p4 = nc.dram_tensor("d_p4", [8, N], F32, addr_space="Shared")
    d_EnT = nc.dram_tensor("d_EnT", [32, N], BF16)
    d_E2T = nc.dram_tensor("d_E2T", [8, N], BF16)
    d_XlT = nc.dram_tensor("d_XlT", [HID, N], BF16)
    d_in1 = nc.dram_tensor("d_in1", [1, N], F32)
    d_in2 = nc.dram_tensor("d_in2", [1, N], F32)

    with tile.TileContext(nc) as tc, ExitStack() as _stk:
        pers = _stk.enter_context(tc.tile_pool(name="pers", bufs=1))

        onr = pers.tile([1, 128], F32, tag="onr")
        onc = pers.tile([128, 1], F32, tag="onc")
        idt = pers.tile([128, 128], BF16, tag="idt")
        nc.sync.dma_start(onr[:], ones_row[:])
        nc.sync.dma_start(onc[:], ones_col[:])
        nc.sync.dma_start(idt[:], ident[:])

        roff = nc.gpsimd.alloc_register("roff")
        nc.gpsimd.reg_load(roff, roff_in[0:1, 0:1])
        roffs = nc.gpsimd.snap(roff)

        wvx_t = pers.tile([128, HC, 8], BF16, tag="wvx_t")
        wve_t = pers.tile([128, HC, 10], BF16, tag="wve_t")
        wvl_t = pers.tile([128, HC, 2], BF16, tag="wvl_t")
        wcat_t = pers.tile([128, HC, 40], BF16, tag="wcat_t")
        nc.sync.dma_start(wvx_t[:], wvx[:])
        nc.sync.dma_start(wve_t[:], wve[:])
        nc.sync.dma_start(wvl_t[:], wvl[:])
        nc.sync.dma_start(wcat_t[:], Wcat[:])

        XhkT = pers.tile([128, HC, R], BF16, tag="XhkT")
        EkT = pers.tile([128, HC, R], BF16, tag="EkT")
        Dv_all = pers.tile([128, NC], F32, tag="Dv_all")
        Dv_col = pers.tile([128, RC], F32, tag="Dv_col")

        # ---------- phase 1: sq / Gram / dist / radius / edge ----------
        with tc.tile_pool(name="p1a", bufs=1) as p1a:
            xt = p1a.tile([128, FC, N], BF16, tag="xt")
            xtk = p1a.tile([128, FC, R], BF16, tag="xtk")
            nc.sync.dma_start(xt[:], XT[:])
            nc.sync.dma_start(xtk[:], XTk[:])
            sq_row = p1a.tile([1, N], F32, tag="sq_row")

            with tc.tile_pool(name="p1sq", bufs=2) as p1sq, \
                 tc.tile_pool(name="ps_sq", bufs=1, space="PSUM") as ps_sq:
                pqs = [ps_sq.tile([1, 512], F32, tag=f"pq{c}", name=f"pq{c}") for c in range(NCC)]
                for kc in range(FC):
                    x2 = p1sq.tile([128, N], F32, tag="x2")
                    nc.vector.scalar_tensor_tensor(
                        out=x2[:], in0=xt[:, kc, :], scalar=1.0,
                        in1=xt[:, kc, :], op0=OP.mult, op1=OP.mult)
                    for c in range(NCC):
                        nc.tensor.matmul(pqs[c][:], onc[:], x2[:, c * 512:(c + 1) * 512],
                                         start=(kc == 0), stop=(kc == FC - 1))
                for c in range(NCC):
                    nc.vector.tensor_copy(sq_row[0:1, c * 512:(c + 1) * 512], pqs[c][:])

            sq_col = p1a.tile([128, RC], F32, tag="sq_col")
            nc.sync.dma_start(d_sq[:], sq_row[:])
            nc.gpsimd.dma_start(
                sq_col[:],
                d_sq[0:1, bass.ds(roffs, R)].rearrange("x (a b) -> x b a", a=RC))

            with tc.tile_pool(name="p1w", bufs=3) as p1w, \
                 tc.tile_pool(name="p1b", bufs=1) as p1b, \
                 tc.tile_pool(name="ps1g", bufs=3, space="PSUM") as ps1g, \
                 tc.tile_pool(name="ps1b", bufs=2, space="PSUM") as ps1b:
                sqb = p1b.tile([128, N], F32, tag="sqb")
                for c in range(NCC):
                    pb = ps1b.tile([128, 512], F32, tag="pb")
                    nc.tensor.matmul(pb[:], onr[:], sq_row[0:1, c * 512:(c + 1) * 512],
                                     start=True, stop=True)
                    nc.vector.tensor_copy(sqb[:, c * 512:(c + 1) * 512], pb[:])

                dist = p1b.tile([128, RC, N], BF16, tag="dist")
                radacc = p1b.tile([128, 32], F32, tag="radacc")
                for mi in range(RC):
                    for c in range(NCC):
                        pg = ps1g.tile([128, 512], F32, tag="pg")
                        for kc in range(FC):
                            nc.tensor.matmul(pg[:], xtk[:, kc, mi * 128:(mi + 1) * 128],
                                             xt[:, kc, c * 512:(c + 1) * 512],
                                             start=(kc == 0), stop=(kc == FC - 1))
                        td = p1w.tile([128, 512], F32, tag="td")
                        nc.vector.scalar_tensor_tensor(
                            out=td[:], in0=pg[:], scalar=-2.0,
                            in1=sqb[:, c * 512:(c + 1) * 512], op0=OP.mult, op1=OP.add)
                        nc.scalar.activation(
                            dist[:, mi, c * 512:(c + 1) * 512], td[:], AF.Abs,
                            bias=sq_col[:, mi:mi + 1], scale=1.0,
                            accum_out=radacc[:, mi * 8 + c:mi * 8 + c + 1])

                rsum = p1w.tile([128, 1], F32, tag="rsum")
                nc.vector.tensor_reduce(rsum[:], radacc[:], axis=AX.X, op=OP.add)
                pt1 = ps1b.tile([128, 512], F32, tag="pt1")
                nc.tensor.matmul(pt1[0:1, 0:1], onc[:], rsum[:], start=True, stop=True)
                rad_sb = p1w.tile([1, 1], F32, tag="rad_sb")
                nc.vector.tensor_copy(rad_sb[:], pt1[0:1, 0:1])
                nc.sync.dma_start(d_rad_i[:], rad_sb[:])
                nc.gpsimd.collective_compute("AllGather", OP.bypass, replica_groups=rg,
                                             ins=[d_rad_i[:].opt()],
                                             outs=[d_rad_o[:].opt()])
                rad8 = p1w.tile([P, 1], F32, tag="rad8")
                nc.sync.dma_start(rad8[:], d_rad_o[:])
                pt2 = ps1b.tile([128, 512], F32, tag="pt1")
                nc.tensor.matmul(pt2[0:1, 0:1], onc[0:P, 0:1], rad8[:],
                                 start=True, stop=True)
                thr1 = p1w.tile([1, 1], F32, tag="thr1")
                nc.vector.tensor_scalar(out=thr1[:], in0=pt2[0:1, 0:1],
                                        scalar1=1.0 / (5.0 * float(N) * float(N)),
                                        scalar2=None, op0=OP.mult)
                pt3 = ps1b.tile([128, 512], F32, tag="pt1")
                nc.tensor.matmul(pt3[0:128, 0:1], onr[:], thr1[:], start=True, stop=True)
                thr_col = p1w.tile([128, 1], F32, tag="thr_col")
                nc.vector.tensor_copy(thr_col[:], pt3[0:128, 0:1])

                for mi in range(RC):
                    e8 = p1w.tile([128, N], FP8, tag="e8")
                    nc.vector.tensor_scalar(out=e8[:], in0=dist[:, mi, :],
                                            scalar1=thr_col[:], scalar2=None, op0=OP.is_lt)
                    nc.sync.dma_start(d_edge_i[mi * 128:(mi + 1) * 128, :], e8[:])
                nc.gpsimd.collective_compute("AllGather", OP.bypass, replica_groups=rg,
                                             ins=[d_edge_i[:].opt()],
                                             outs=[d_edge[:].opt()])

        # ---------- phase 2: H = edge_k @ edge > 0 ----------
        with tc.tile_pool(name="p2a", bufs=1) as p2a, \
             tc.tile_pool(name="p2w", bufs=3) as p2w, \
             tc.tile_pool(name="ps2", bufs=4, space="PSUM") as ps2:
            ef = p2a.tile([128, NC, N], FP8, tag="ef")
            ec = p2a.tile([128, NC, R], FP8, tag="ec")
            nc.sync.dma_start(ef[:], d_edge.rearrange("(jc p) j -> p jc j", p=128))
            nc.gpsimd.dma_start(
                ec[:],
                d_edge.rearrange("(jc p) j -> p jc j", p=128)[:, :, bass.ds(roffs, R)])
            degacc = p2a.tile([128, 32], F32, tag="degacc")
            for mi in range(RC):
                for c in range(NCC):
                    ph = ps2.tile([128, 512], F32, tag="ph")
                    for jp in range(NC // 2):
                        nc.tensor.matmul(
                            ph[:], ec[:, 2 * jp:2 * jp + 2, mi * 128:(mi + 1) * 128],
                            ef[:, 2 * jp:2 * jp + 2, c * 512:(c + 1) * 512],
                            start=(jp == 0), stop=(jp == NC // 2 - 1), perf_mode=DR)
                    hb = p2w.tile([128, 512], BF16, tag="hb")
                    nc.vector.tensor_scalar(out=hb[:], in0=ph[:], scalar1=0.0,
                                            scalar2=0.0, op0=OP.is_gt, op1=OP.add,
                                            accum_out=degacc[:, mi * 8 + c:mi * 8 + c + 1])
                    nc.sync.dma_start(
                        d_Hk[mi * 128:(mi + 1) * 128, c * 512:(c + 1) * 512], hb[:])
                    h8 = p2w.tile([128, 512], FP8, tag="h8")
                    nc.scalar.activation(h8[:], ph[:], AF.Sign)
                    nc.sync.dma_start(
                        d_H_i[mi * 128:(mi + 1) * 128, c * 512:(c + 1) * 512], h8[:])
                dg = p2w.tile([128, 1], F32, tag="dg")
                nc.vector.tensor_reduce(dg[:], degacc[:, mi * 8:(mi + 1) * 8], axis=AX.X,
                                        op=OP.add)
                rdg = p2w.tile([128, 1], F32, tag="rdg")
                nc.vector.reciprocal(rdg[:], dg[:])
                nc.scalar.activation(Dv_col[:, mi:mi + 1], rdg[:], AF.Sqrt)
            nc.sync.dma_start(d_Dv_i.rearrange("(b a) x -> a (b x)", b=RC), Dv_col[:])
            nc.gpsimd.collective_compute("AllGather", OP.bypass, replica_groups=rg,
                                         ins=[d_Dv_i[:].opt()], outs=[d_Dv[:].opt()])
            nc.gpsimd.collective_compute("AllGather", OP.bypass, replica_groups=rg,
                                         ins=[d_H_i[:].opt()], outs=[d_H[:].opt()])
            nc.sync.dma_start(Dv_all[:], d_Dv.rearrange("(jc p) x -> p (jc x)", p=128))
        Hc = pers.tile([128, NC, R], FP8, tag="Hc")
        nc.gpsimd.dma_start(
            Hc[:], d_H.rearrange("(jc p) j -> p jc j", p=128)[:, :, bass.ds(roffs, R)])

        # ---------- phase 3: V, U, E ----------
        with tc.tile_pool(name="p3a", bufs=1) as p3a, \
             tc.tile_pool(name="p3w", bufs=2) as p3w, \
             tc.tile_pool(name="ps3", bufs=2, space="PSUM") as ps3:
            V = p3a.tile([128, NC, F], BF16, tag="V")
            nc.sync.dma_start(V[:], Xn[:])
            for jc in range(NC):
                nc.vector.tensor_scalar(out=V[:, jc, :], in0=V[:, jc, :],
                                        scalar1=Dv_all[:, jc:jc + 1], scalar2=None,
                                        op0=OP.mult)
            Usc = p3a.tile([128, RC, F], BF16, tag="Usc")
            for mi in range(RC):
                pu = ps3.tile([128, 512], F32, tag="acc")
                for jc in range(NC):
                    nc.tensor.matmul(pu[:], Hc[:, jc, mi * 128:(mi + 1) * 128],
                                     V[:, jc, :], start=(jc == 0), stop=(jc == NC - 1))
                nc.vector.tensor_scalar(out=Usc[:, mi, :], in0=pu[:],
                                        scalar1=Dv_col[:, mi:mi + 1], scalar2=None,
                                        op0=OP.mult)
            UT = p3a.tile([128, FC, R], BF16, tag="UT")
            for mi in range(RC):
                for fc in range(FC):
                    pt = ps3.tile([128, 128], BF16, tag="pt")
                    nc.tensor.transpose(pt[:], Usc[:, mi, fc * 128:(fc + 1) * 128], idt[:])
                    nc.vector.tensor_copy(UT[:, fc, mi * 128:(mi + 1) * 128], pt[:])
            tht = p3a.tile([128, FC, HID], BF16, tag="tht")
            nc.sync.dma_start(tht[:], theta_t[:])
            for hc in range(HC):
                pe = ps3.tile([128, 512], F32, tag="acc")
                for fc in range(FC):
                    nc.tensor.matmul(pe[:], tht[:, fc, hc * 128:(hc + 1) * 128],
                                     UT[:, fc, :], start=(fc == 0), stop=(fc == FC - 1))
                nc.vector.tensor_copy(EkT[:, hc, :], pe[:])
            Enat = p3a.tile([128, RC, HID], BF16, tag="Enat")
            for hc in range(HC):
                for mi in range(RC):
                    pt = ps3.tile([128, 128], BF16, tag="pt")
                    nc.tensor.transpose(pt[:], EkT[:, hc, mi * 128:(mi + 1) * 128], idt[:])
                    nc.vector.tensor_copy(Enat[:, mi, hc * 128:(hc + 1) * 128], pt[:])
            nc.sync.dma_start(d_En_i.rearrange("(b a) h -> a b h", b=RC), Enat[:])
            nc.gpsimd.collective_compute("AllGather", OP.bypass, replica_groups=rg,
                                         ins=[d_En_i[:].opt()], outs=[d_En[:].opt()])

        # ---------- phase 4: Xh, then xx ----------
        pmid = _stk.enter_context(tc.tile_pool(name="pmid", bufs=1))
        with tc.tile_pool(name="p4a", bufs=1) as p4a, \
             tc.tile_pool(name="p4w", bufs=1) as p4w, \
             tc.tile_pool(name="p4s", bufs=3) as p4s, \
             tc.tile_pool(name="ps4", bufs=2, space="PSUM") as ps4:
            with tc.tile_pool(name="p4d", bufs=1) as p4d:
                DeE = p4d.tile([128, NC, HID], BF16, tag="DeE")
                nc.sync.dma_start(DeE[:], d_En.rearrange("(jc p) h -> p jc h", p=128))
                for jc in range(NC):
                    nc.vector.tensor_scalar(out=DeE[:, jc, :], in0=DeE[:, jc, :],
                                            scalar1=Dv_all[:, jc:jc + 1], scalar2=None,
                                            op0=OP.mult)
                Xhn = p4d.tile([128, RC, HID], BF16, tag="Xhn")
                for mi in range(RC):
                    px = ps4.tile([128, 512], F32, tag="acc")
                    for jc in range(NC):
                        nc.tensor.matmul(px[0:128, 0:HID],
                                         Hc[:, jc, mi * 128:(mi + 1) * 128],
                                         DeE[:, jc, :], start=(jc == 0),
                                         stop=(jc == NC - 1))
                    nc.vector.tensor_scalar(out=Xhn[:, mi, :], in0=px[0:128, 0:HID],
                                            scalar1=Dv_col[:, mi:mi + 1], scalar2=None,
                                            op0=OP.mult)
                for hc in range(HC):
                    for mi in range(RC):
                        pt = ps4.tile([128, 128], BF16, tag="pt")
                        nc.tensor.transpose(pt[:], Xhn[:, mi, hc * 128:(hc + 1) * 128],
                                            idt[:])
                        nc.vector.tensor_copy(XhkT[:, hc, mi * 128:(mi + 1) * 128], pt[:])
            nc.sync.dma_start(d_XhT_i.rearrange("(b a) h -> a b h", b=HC), XhkT[:])
            nc.gpsimd.collective_compute("AllGather", OP.bypass, replica_groups=rg,
                                         ins=[d_XhT_i[:].opt()], outs=[d_XhT[:].opt()])
            # batched v-vector locals: rows 0..4 from EkT (v1_h, v3), 5..8 from XhkT (v2_h)
            vE = p4w.tile([5, R], F32, tag="vE")
            pv9 = ps4.tile([128, 512], F32, tag="acc")
            for hc in range(HC):
                nc.tensor.matmul(pv9[0:5, :], wve_t[:, hc, 0:5], EkT[:, hc, :],
                                 start=(hc == 0), stop=(hc == HC - 1))
            nc.vector.tensor_copy(vE[:], pv9[0:5, :])
            vX = p4w.tile([4, R], F32, tag="vX")
            pv4 = ps4.tile([128, 512], F32, tag="acc")
            for hc in range(HC):
                nc.tensor.matmul(pv4[0:4, :], wvx_t[:, hc, 4:8], XhkT[:, hc, :],
                                 start=(hc == 0), stop=(hc == HC - 1))
            nc.vector.tensor_copy(vX[:], pv4[0:4, :])
            nc.sync.dma_start(d_v1_i[0:5, :], vE[:])
            nc.sync.dma_start(d_v1_i[5:9, :], vX[:])
            nc.gpsimd.collective_compute("AllGather", OP.bypass, replica_groups=rg,
                                         ins=[d_v1_i[:].opt()], outs=[d_v1[:].opt()])

            xx = pmid.tile([128, RC, N], FP8, tag="xx")
            Hf = p4a.tile([128, NC, N], FP8, tag="Hf")
            nc.sync.dma_start(Hf[:], d_H.rearrange("(jc p) j -> p jc j", p=128))
            for mi in range(RC):
                sel = p4w.tile([128, N], BF16, tag="sel")
                nc.sync.dma_start(sel[:], selbig[:, mi, :])
                for c in range(NCC):
                    pxx = ps4.tile([128, 512], F32, tag="acc2", bufs=4)
                    for jp in range(NC // 2):
                        nc.tensor.matmul(
                            pxx[:], Hc[:, 2 * jp:2 * jp + 2, mi * 128:(mi + 1) * 128],
                            Hf[:, 2 * jp:2 * jp + 2, c * 512:(c + 1) * 512],
                            start=(jp == 0), stop=(jp == NC // 2 - 1), perf_mode=DR)
                    xv = p4s.tile([128, 512], F32, tag="xv")
                    nc.vector.scalar_tensor_tensor(
                        out=xv[:], in0=pxx[:], scalar=1.0,
                        in1=sel[:, c * 512:(c + 1) * 512], op0=OP.mult, op1=OP.add)
                    nc.vector.tensor_scalar(out=xx[:, mi, c * 512:(c + 1) * 512],
                                            in0=xv[:], scalar1=0.0, scalar2=None,
                                            op0=OP.is_gt)


        xx = xx  # noqa: F821  (assigned in phase 4 scope above)

        # ---------- shared helpers ----------
        def inv_norm_row(mT, dram_row, invc_out, pool, psw, tag):
            """mT [128, HC, N] -> inverse column norms [1, N] f32 (returned tile)
            and the local per-partition slices invc_out [128, RC]."""
            s2 = pool.tile([128, HC, N], F32, tag=tag + "s2", bufs=1)
            for hc in range(HC):
                nc.vector.scalar_tensor_tensor(out=s2[:, hc, :], in0=mT[:, hc, :],
                                               scalar=1.0, in1=mT[:, hc, :],
                                               op0=OP.mult, op1=OP.mult)
            n2r = pool.tile([1, N], F32, tag=tag + "r", bufs=1)
            for c in range(NCC):
                pr = psw.tile([128, 512], F32, tag="acc")
                for hc in range(HC):
                    nc.tensor.matmul(pr[0:1, :], onc[:], s2[:, hc, c * 512:(c + 1) * 512],
                                     start=(hc == 0), stop=(hc == HC - 1))
                nc.vector.tensor_copy(n2r[0:1, c * 512:(c + 1) * 512], pr[0:1, :])
            rn = pool.tile([1, N], F32, tag=tag + "rn", bufs=1)
            nc.vector.reciprocal(rn[:], n2r[:])
            nc.scalar.activation(n2r[:], rn[:], AF.Sqrt)
            nc.sync.dma_start(dram_row[:], n2r[:])
            nc.gpsimd.dma_start(
                invc_out,
                dram_row[0:1, bass.ds(roffs, R)].rearrange("x (a b) -> x b a", a=RC))
            return n2r

        def rho_pass(kT, fT, invc, invr, rho_dram_i, rho_dram_o, pool, psw, tag):
            invb = pool.tile([128, N], BF16, tag=tag + "invb", bufs=1)
            for c in range(NCC):
                pb = psw.tile([128, 512], F32, tag="bc")
                nc.tensor.matmul(pb[:], onr[:], invr[0:1, c * 512:(c + 1) * 512],
                                 start=True, stop=True)
                nc.vector.tensor_copy(invb[:, c * 512:(c + 1) * 512], pb[:])
            racc = pool.tile([128, 32], F32, tag=tag + "acc")
            for mi in range(RC):
                for c in range(NCC):
                    pC = psw.tile([128, 512], F32, tag="acc")
                    for hc in range(HC):
                        nc.tensor.matmul(pC[:], kT[:, hc, mi * 128:(mi + 1) * 128],
                                         fT[:, hc, c * 512:(c + 1) * 512],
                                         start=(hc == 0), stop=(hc == HC - 1))
                    q = pool.tile([128, 512], BF16, tag=tag + "q")
                    nc.vector.scalar_tensor_tensor(out=q[:], in0=pC[:],
                                                   scalar=invc[:, mi:mi + 1],
                                                   in1=invb[:, c * 512:(c + 1) * 512],
                                                   op0=OP.mult, op1=OP.mult)
                    t = pool.tile([128, 512], BF16, tag=tag + "t")
                    nc.vector.tensor_scalar(out=t[:], in0=q[:], scalar1=SIGMA,
                                            scalar2=None, op0=OP.is_gt)
                    xc = pool.tile([128, 512], BF16, tag=tag + "xc")
                    nc.vector.tensor_copy(xc[:], xx[:, mi, c * 512:(c + 1) * 512])
                    tx = pool.tile([128, 512], BF16, tag=tag + "tx")
                    nc.vector.tensor_tensor(tx[:], t[:], xc[:], op=OP.mult)
                    nc.vector.scalar_tensor_tensor(
                        out=t[:], in0=q[:], scalar=1.0, in1=tx[:], op0=OP.mult,
                        op1=OP.mult, accum_out=racc[:, mi * 8 + c:mi * 8 + c + 1])
            rloc = pool.tile([128, RC], F32, tag=tag + "rl")
            for mi in range(RC):
                nc.vector.tensor_reduce(rloc[:, mi:mi + 1], racc[:, mi * 8:(mi + 1) * 8],
                                        axis=AX.X, op=OP.add)
            nc.sync.dma_start(rho_dram_i.rearrange("(b a) x -> a (b x)", b=RC), rloc[:])
            nc.gpsimd.collective_compute("AllGather", OP.bypass, replica_groups=rg,
                                         ins=[rho_dram_i[:].opt()],
                                         outs=[rho_dram_o[:].opt()])

        def rho_e_pass(rho_dram, re_dram_i, re_dram_o, pool, psw, tag):
            rb32 = pool.tile([128, NC], F32, tag=tag + "b32")
            nc.sync.dma_start(rb32[:], rho_dram.rearrange("(jc p) x -> p (jc x)", p=128))
            rbf = pool.tile([128, NC], BF16, tag=tag + "bf")
            nc.vector.tensor_copy(rbf[:], rb32[:])
            rec = pool.tile([128, RC], F32, tag=tag + "rec")
            for mi in range(RC):
                pre = psw.tile([128, 512], F32, tag="acc")
                for jc in range(NC):
                    nc.tensor.matmul(pre[0:128, 0:1], Hc[:, jc, mi * 128:(mi + 1) * 128],
                                     rbf[:, jc:jc + 1], start=(jc == 0),
                                     stop=(jc == NC - 1))
                nc.vector.tensor_copy(rec[:, mi:mi + 1], pre[0:128, 0:1])
            nc.sync.dma_start(re_dram_i.rearrange("(b a) x -> a (b x)", b=RC), rec[:])
            nc.gpsimd.collective_compute("AllGather", OP.bypass, replica_groups=rg,
                                         ins=[re_dram_i[:].opt()],
                                         outs=[re_dram_o[:].opt()])

        def flag_scale_from(rho_dram, pool, tag):
            mx8 = pool.tile([1, NCC], F32, tag=tag + "mx8")
            mn8 = pool.tile([1, NCC], F32, tag=tag + "mn8")
            for c in range(NCC):
                rr = pool.tile([1, 512], F32, tag=tag + "row")
                nc.sync.dma_start(rr[:],
                                  rho_dram.rearrange("a x -> x a")[0:1,
                                                                  c * 512:(c + 1) * 512])
                nc.vector.tensor_reduce(mx8[0:1, c:c + 1], rr[:], axis=AX.X, op=OP.max)
                nc.vector.tensor_reduce(mn8[0:1, c:c + 1], rr[:], axis=AX.X, op=OP.min)
            mx = pool.tile([1, 1], F32, tag=tag + "mx")
            mn = pool.tile([1, 1], F32, tag=tag + "mn")
            nc.vector.tensor_reduce(mx[:], mx8[:], axis=AX.X, op=OP.max)
            nc.vector.tensor_reduce(mn[:], mn8[:], axis=AX.X, op=OP.min)
            fl = pool.tile([1, 1], F32, tag=tag + "fl")
            nc.vector.tensor_tensor(fl[:], mx[:], mn[:], op=OP.is_equal)
            nc.vector.tensor_scalar(out=fl[:], in0=fl[:], scalar1=-1.0, scalar2=1.0,
                                    op0=OP.mult, op1=OP.add)
            return fl

        def scale_after_elu(dst, src_dram, flag_dram, nparts, pool, psw, tag,
                            dst_dram=None, dst_dtype=None):
            """dst[nparts, N] = elu(AR result) * (1 - degenerate_flag).
            If dst_dram given, stream chunks there instead (dst ignored)."""
            fl = flag_scale_from(flag_dram, pool, tag + "f")
            pf = psw.tile([128, 512], F32, tag="bc")
            nc.tensor.matmul(pf[0:nparts, 0:1], onr[0:1, 0:nparts], fl[:],
                             start=True, stop=True)
            fc = pool.tile([nparts, 1], F32, tag=tag + "fc")
            nc.vector.tensor_copy(fc[:], pf[0:nparts, 0:1])
            for c in range(NCC):
                cs = slice(c * 512, (c + 1) * 512)
                ar = pool.tile([nparts, 512], F32, tag=tag + "ar")
                nc.sync.dma_start(ar[:], src_dram[:, cs])
                r_ = pool.tile([nparts, 512], F32, tag=tag + "r")
                nc.scalar.activation(r_[:], ar[:], AF.Relu)
                zmr = pool.tile([nparts, 512], F32, tag=tag + "z")
                nc.vector.scalar_tensor_tensor(out=zmr[:], in0=ar[:], scalar=1.0,
                                               in1=r_[:], op0=OP.mult, op1=OP.subtract)
                ex = pool.tile([nparts, 512], F32, tag=tag + "e")
                nc.scalar.activation(ex[:], zmr[:], AF.Exp)
                el = pool.tile([nparts, 512], F32, tag=tag + "el")
                nc.vector.scalar_tensor_tensor(out=el[:], in0=ex[:], scalar=-1.0,
                                               in1=r_[:], op0=OP.add, op1=OP.add)
                if dst_dram is None:
                    nc.vector.tensor_scalar(out=dst[:, cs], in0=el[:], scalar1=fc[:],
                                            scalar2=None, op0=OP.mult)
                else:
                    oc = pool.tile([nparts, 512], dst_dtype, tag=tag + "oc")
                    nc.vector.tensor_scalar(out=oc[:], in0=el[:], scalar1=fc[:],
                                            scalar2=None, op0=OP.mult)
                    nc.sync.dma_start(dst_dram[:, cs], oc[:])

        def fetch_vrow(v_dram, nv, idx, pool, tag):
            vr = pool.tile([1, N], F32, tag="vrow", name=tag, bufs=1)
            for c in range(P):
                nc.sync.dma_start(vr[0:1, c * R:(c + 1) * R],
                                  v_dram[c * nv + idx:c * nv + idx + 1, :])
            return vr

        def ucol_mm(wtile, col, locT, pool, psw, tag):
            uc = pool.tile([128, RC], F32, tag=tag)
            for mi in range(RC):
                pu = psw.tile([128, 512], F32, tag="acc")
                for hc in range(HC):
                    nc.tensor.matmul(pu[0:128, 0:1], locT[:, hc, mi * 128:(mi + 1) * 128],
                                     wtile[:, hc, col:col + 1],
                                     start=(hc == 0), stop=(hc == HC - 1))
                nc.vector.tensor_copy(uc[:, mi:mi + 1], pu[0:128, 0:1])
            return uc

        plate = _stk.enter_context(tc.tile_pool(name="plate", bufs=1))
        Hkb = plate.tile([128, RC, N], BF16, tag="Hkb")
        nc.sync.dma_start(Hkb[:], d_Hk.rearrange("(b a) j -> a b j", b=RC))

        def attention(u_col, v_row, agg, agg_off, dst_dram, dst_row0, pool, psw, tag):
            """Masked softmax attention for local rows; DMAs the partial
            (att^T @ agg') into dst_dram[dst_row0:dst_row0+8, :] (f32)."""
            vmax = pool.tile([1, 1], F32, tag=tag + "vm")
            nc.vector.tensor_reduce(vmax[:], v_row[:], axis=AX.X, op=OP.max)
            pvb = psw.tile([128, 512], F32, tag="bc")
            nc.tensor.matmul(pvb[0:128, 0:1], onr[:], vmax[:], start=True, stop=True)
            vmc = pool.tile([128, 1], F32, tag=tag + "vmc")
            nc.vector.tensor_copy(vmc[:], pvb[0:128, 0:1])
            nb = pool.tile([128, RC], F32, tag=tag + "nb")
            for mi in range(RC):
                nc.scalar.activation(nb[:, mi:mi + 1], u_col[:, mi:mi + 1], AF.Prelu,
                                     bias=vmc[:], scale=1.0, alpha=0.2)
            nc.vector.tensor_scalar(out=nb[:], in0=nb[:], scalar1=-1.0, scalar2=None,
                                    op0=OP.mult)
            Vb = pool.tile([128, N], BF16, tag=tag + "Vb", bufs=1)
            for c in range(NCC):
                pbb = psw.tile([128, 512], F32, tag="bc")
                nc.tensor.matmul(pbb[:], onr[:], v_row[0:1, c * 512:(c + 1) * 512],
                                 start=True, stop=True)
                nc.vector.tensor_copy(Vb[:, c * 512:(c + 1) * 512], pbb[:])
            em = pool.tile([128, RC, N], BF16, tag=tag + "em", bufs=1)
            ag = pool.tile([128, RC, 8], BF16, tag=tag + "ag")
            racc = pool.tile([128, RC, 8], F32, tag=tag + "racc")
            # batch all Prelu score passes (single ACT table context), scores
            # land in em in-place ...
            for mi in range(RC):
                for c in range(NCC):
                    nc.scalar.activation(em[:, mi, c * 512:(c + 1) * 512],
                                         Vb[:, c * 512:(c + 1) * 512], AF.Prelu,
                                         bias=u_col[:, mi:mi + 1], scale=1.0, alpha=0.2)
            # ... then all Exp passes + masking (overwrites em chunk-by-chunk)
            for mi in range(RC):
                for c in range(NCC):
                    e0 = pool.tile([128, 512], BF16, tag=tag + "e0", bufs=3)
                    nc.scalar.activation(e0[:], em[:, mi, c * 512:(c + 1) * 512],
                                         AF.Exp, bias=nb[:, mi:mi + 1], scale=1.0)
                    nc.vector.scalar_tensor_tensor(
                        out=em[:, mi, c * 512:(c + 1) * 512], in0=e0[:], scalar=1.0,
                        in1=Hkb[:, mi, c * 512:(c + 1) * 512], op0=OP.mult, op1=OP.mult,
                        accum_out=racc[:, mi, c:c + 1])
            for mi in range(RC):
                rc_ = pool.tile([128, 1], F32, tag=tag + "rc")
                nc.vector.tensor_reduce(rc_[:], racc[:, mi, :], axis=AX.X, op=OP.add)
                ir = pool.tile([128, 1], F32, tag=tag + "ir")
                nc.vector.reciprocal(ir[:], rc_[:])
                nc.vector.tensor_scalar(out=ag[:, mi, :],
                                        in0=agg[:, mi, agg_off:agg_off + 8],
                                        scalar1=ir[:], scalar2=None, op0=OP.mult)
            for c in range(NCC):
                pat = psw.tile([8, 512], F32, tag="pat")
                for mi in range(RC):
                    nc.tensor.matmul(pat[:], ag[:, mi, :],
                                     em[:, mi, c * 512:(c + 1) * 512],
                                     start=(mi == 0), stop=(mi == RC - 1))
                pc_sb = pool.tile([8, 512], F32, tag=tag + "pc")
                nc.scalar.copy(pc_sb[:], pat[:])
                nc.sync.dma_start(
                    dst_dram[dst_row0:dst_row0 + 8, c * 512:(c + 1) * 512], pc_sb[:])

        # ---------- phase 5: rho1 / rho_e1 ----------
        with tc.tile_pool(name="p5", bufs=2) as p5, \
             tc.tile_pool(name="p5b", bufs=1) as p5b, \
             tc.tile_pool(name="ps5", bufs=2, space="PSUM") as ps5:
            XhT = p5b.tile([128, HC, N], BF16, tag="XhT")
            for c in range(P):
                for hc in range(HC):
                    nc.sync.dma_start(
                        XhT[:, hc, c * R:(c + 1) * R],
                        d_XhT[c * HID + hc * 128:c * HID + (hc + 1) * 128, :])
            invc1 = plate.tile([128, RC], F32, tag="invc1")
            invr1 = inv_norm_row(XhT, d_in1, invc1[:], p5b, ps5, "in1")
            rho_pass(XhkT, XhT, invc1, invr1, d_rho1_i, d_rho1, p5, ps5, "r1")
            rho_e_pass(d_rho1, d_re1_i, d_re1, p5, ps5, "re1")

        # ---------- phase 6: attention-1 for 4 heads -> EnewT ----------
        with tc.tile_pool(name="p6", bufs=2) as p6, \
             tc.tile_pool(name="p6p", bufs=1) as p6p, \
             tc.tile_pool(name="ps6", bufs=2, space="PSUM") as ps6:
            WXk = p6p.tile([128, RC, 32], BF16, tag="WXk")
            for h in range(4):
                for mi in range(RC):
                    pw = ps6.tile([128, 512], F32, tag="acc")
                    for hc in range(HC):
                        nc.tensor.matmul(pw[0:128, 0:8],
                                         XhkT[:, hc, mi * 128:(mi + 1) * 128],
                                         wcat_t[:, hc, h * 8:(h + 1) * 8],
                                         start=(hc == 0), stop=(hc == HC - 1))
                    nc.vector.tensor_copy(WXk[:, mi, h * 8:(h + 1) * 8], pw[0:128, 0:8])
            for h in range(4):
                u1 = ucol_mm(wvx_t, h, XhkT, p6, ps6, "u1")
                v1 = fetch_vrow(d_v1, 9, h, p6, "v1")
                attention(u1, v1, WXk, h * 8, d_p1_i, h * 8, p6, ps6, "a1")
            nc.gpsimd.collective_compute("AllReduce", OP.add, replica_groups=rg,
                                         ins=[d_p1_i[:].opt()], outs=[d_p1[:].opt()])
            scale_after_elu(None, d_p1, d_rho1, 32, p6p, ps6, "s1",
                            dst_dram=d_EnT, dst_dtype=BF16)

        E2n = plate.tile([128, RC, 32], BF16, tag="E2n")
        with tc.tile_pool(name="p6b", bufs=2) as p6b, \
             tc.tile_pool(name="ps6b", bufs=2, space="PSUM") as ps6b:
            enl = p6b.tile([32, R], BF16, tag="enl")
            nc.gpsimd.dma_start(enl[:], d_EnT[:, bass.ds(roffs, R)])
            for mi in range(RC):
                pt = ps6b.tile([128, 128], BF16, tag="pt")
                nc.tensor.transpose(pt[0:128, 0:32], enl[:, mi * 128:(mi + 1) * 128],
                                    idt[0:32, 0:32])
                nc.vector.tensor_copy(E2n[:, mi, :], pt[0:128, 0:32])

        # ---------- phase 7: attention-2 -> XcT -> XlT ----------
        with tc.tile_pool(name="p7", bufs=2) as p7, \
             tc.tile_pool(name="p7p", bufs=1) as p7p, \
             tc.tile_pool(name="ps7", bufs=2, space="PSUM") as ps7:
            for h in range(4):
                u2 = ucol_mm(wve_t, 5 + h, EkT, p7, ps7, "u2")
                v2 = fetch_vrow(d_v1, 9, 5 + h, p7, "v2")
                attention(u2, v2, E2n, h * 8, d_p2_i, h * 8, p7, ps7, "a2")
            nc.gpsimd.collective_compute("AllReduce", OP.add, replica_groups=rg,
                                         ins=[d_p2_i[:].opt()], outs=[d_p2[:].opt()])
            XcT = p7p.tile([32, N], BF16, tag="XcT")
            scale_after_elu(XcT[:], d_p2, d_re1, 32, p7p, ps7, "s2")

            lw = p7p.tile([32, HID], BF16, tag="lw")
            nc.sync.dma_start(lw[:], linw[:])
            lb = p7p.tile([128, HC], F32, tag="lb")
            nc.sync.dma_start(lb[:], linb[:])
            for hc in range(HC):
                for c in range(NCC):
                    pxl = ps7.tile([128, 512], F32, tag="acc")
                    nc.tensor.matmul(pxl[:], lw[:, hc * 128:(hc + 1) * 128],
                                     XcT[:, c * 512:(c + 1) * 512], start=True, stop=True)
                    r_ = p7.tile([128, 512], F32, tag="xlr")
                    nc.scalar.activation(r_[:], pxl[:], AF.Relu, bias=lb[:, hc:hc + 1],
                                         scale=1.0)
                    zmr = p7.tile([128, 512], F32, tag="xlz")
                    nc.vector.scalar_tensor_tensor(out=zmr[:], in0=pxl[:],
                                                   scalar=lb[:, hc:hc + 1], in1=r_[:],
                                                   op0=OP.add, op1=OP.subtract)
                    ex = p7.tile([128, 512], F32, tag="xle")
                    nc.scalar.activation(ex[:], zmr[:], AF.Exp)
                    xlo = p7.tile([128, 512], BF16, tag="xlo")
                    nc.vector.scalar_tensor_tensor(out=xlo[:], in0=ex[:], scalar=-1.0,
                                                   in1=r_[:], op0=OP.add, op1=OP.add)
                    nc.sync.dma_start(
                        d_XlT.rearrange("(b p) j -> p b j", p=128)[:, hc,
                                                                  c * 512:(c + 1) * 512],
                        xlo[:])

        XlkT = plate.tile([128, HC, R], BF16, tag="XlkT")
        nc.gpsimd.dma_start(
            XlkT[:],
            d_XlT.rearrange("(b p) j -> p b j", p=128)[:, :, bass.ds(roffs, R)])

        # ---------- phase 8: rho2 / rho_e2 + v4 ----------
        with tc.tile_pool(name="p8", bufs=2) as p8, \
             tc.tile_pool(name="p8b", bufs=1) as p8b, \
             tc.tile_pool(name="ps8", bufs=2, space="PSUM") as ps8:
            XlT = p8b.tile([128, HC, N], BF16, tag="XlT")
            nc.sync.dma_start(XlT[:], d_XlT.rearrange("(b p) j -> p b j", p=128))
            vl = p8.tile([1, R], F32, tag="vl")
            pvl = ps8.tile([128, 512], F32, tag="acc")
            for hc in range(HC):
                nc.tensor.matmul(pvl[0:1, :], wvl_t[:, hc, 1:2], XlkT[:, hc, :],
                                 start=(hc == 0), stop=(hc == HC - 1))
            nc.vector.tensor_copy(vl[:], pvl[0:1, :])
            nc.sync.dma_start(d_v2_i[:], vl[:])
            nc.gpsimd.collective_compute("AllGather", OP.bypass, replica_groups=rg,
                                         ins=[d_v2_i[:].opt()], outs=[d_v2[:].opt()])
            invc2 = plate.tile([128, RC], F32, tag="invc2")
            invr2 = inv_norm_row(XlT, d_in2, invc2[:], p8b, ps8, "in2")
            rho_pass(XlkT, XlT, invc2, invr2, d_rho2_i, d_rho2, p8, ps8, "r2")
            rho_e_pass(d_rho2, d_re2_i, d_re2, p8, ps8, "re2")

        # ---------- phase 9: final attentions ----------
        with tc.tile_pool(name="p9", bufs=2) as p9, \
             tc.tile_pool(name="p9p", bufs=1) as p9p, \
             tc.tile_pool(name="ps9", bufs=2, space="PSUM") as ps9:
            WX2 = p9p.tile([128, RC, 8], BF16, tag="WX2")
            for mi in range(RC):
                pw = ps9.tile([128, 512], F32, tag="acc")
                for hc in range(HC):
                    nc.tensor.matmul(pw[0:128, 0:8], XlkT[:, hc, mi * 128:(mi + 1) * 128],
                                     wcat_t[:, hc, 32:40],
                                     start=(hc == 0), stop=(hc == HC - 1))
                nc.vector.tensor_copy(WX2[:, mi, :], pw[0:128, 0:8])
            u3 = ucol_mm(wvl_t, 0, XlkT, p9, ps9, "u3")
            v3 = fetch_vrow(d_v1, 9, 4, p9, "v3")
            attention(u3, v3, WX2, 0, d_p3_i, 0, p9, ps9, "a3")
            nc.gpsimd.collective_compute("AllReduce", OP.add, replica_groups=rg,
                                         ins=[d_p3_i[:].opt()], outs=[d_p3[:].opt()])
            scale_after_elu(None, d_p3, d_rho2, 8, p9p, ps9, "s3",
                            dst_dram=d_E2T, dst_dtype=BF16)
            e2l = p9.tile([8, R], BF16, tag="e2l")
            nc.gpsimd.dma_start(e2l[:], d_E2T[:, bass.ds(roffs, R)])
            E3n = p9p.tile([128, RC, 8], BF16, tag="E3n")
            for mi in range(RC):
                pt = ps9.tile([128, 128], BF16, tag="pt")
                nc.tensor.transpose(pt[0:128, 0:8], e2l[:, mi * 128:(mi + 1) * 128],
                                    idt[0:8, 0:8])
                nc.vector.tensor_copy(E3n[:, mi, :], pt[0:128, 0:8])
            u4 = ucol_mm(wve_t, 9, EkT, p9, ps9, "u4")
            v4 = fetch_vrow(d_v2, 1, 0, p9, "v4")
            attention(u4, v4, E3n, 0, d_p4_i, 0, p9, ps9, "a4")
            nc.gpsimd.collective_compute("AllReduce", OP.add, replica_groups=rg,
                                         ins=[d_p4_i[:].opt()], outs=[d_p4[:].opt()])
            scale_after_elu(None, d_p4, d_re2, 8, p9p, ps9, "s3",
                            dst_dram=out_ext, dst_dtype=F32)
    nc.finalize()
    return nc


_BUILT = None


def _prep_inputs(X, theta, W_heads, ax_heads, ae_heads, lin_w, lin_b, W2, ax2, ae2):
    bf = ml_dtypes.bfloat16
    X = np.asarray(X, np.float32)
    Xb = X.astype(bf)
    XTh = np.ascontiguousarray(Xb.T)                             # [F, N]
    XT_t = XTh.reshape(FC, 128, N).transpose(1, 0, 2).copy()     # [128, FC, N]
    Xn_t = Xb.reshape(NC, 128, F).transpose(1, 0, 2).copy()      # [128, NC, F]
    th_t = (np.asarray(theta, np.float32).astype(bf)
            .reshape(FC, 128, HID).transpose(1, 0, 2).copy())
    Wl = [np.asarray(W_heads[i], np.float32) for i in range(4)] + \
         [np.asarray(W2, np.float32)]
    Wcat_f = np.concatenate(Wl, axis=1)                          # [HID, 40]
    Wcat_t = Wcat_f.astype(bf).reshape(HC, 128, 40).transpose(1, 0, 2).copy()

    axl = [np.asarray(a, np.float32) for a in ax_heads]
    ael = [np.asarray(a, np.float32) for a in ae_heads]
    ax2 = np.asarray(ax2, np.float32)
    ae2 = np.asarray(ae2, np.float32)
    # wvx (Xh-side): cols 0..3 u of att1 (W@ax[:D]);  4..7 v of att2 (W@ae[D:])
    wvx_cols = [Wl[i] @ axl[i][:D] for i in range(4)] + \
               [Wl[i] @ ael[i][D:] for i in range(4)]
    # wve (E-side): cols 0..3 v of att1 (W@ax[D:]); col 4 v3 (W2@ax2[D:]);
    #               cols 5..8 u of att2 (W@ae[:D]); col 9 u4 (W2@ae2[:D])
    wve_cols = [Wl[i] @ axl[i][D:] for i in range(4)] + \
               [Wl[4] @ ax2[D:]] + \
               [Wl[i] @ ael[i][:D] for i in range(4)] + \
               [Wl[4] @ ae2[:D]]
    # wvl (Xl-side): col 0 u3 (W2@ax2[:D]); col 1 v4 (W2@ae2[D:])
    wvl_cols = [Wl[4] @ ax2[:D], Wl[4] @ ae2[D:]]
    wvx_f = np.concatenate(wvx_cols, axis=1)
    wve_f = np.concatenate(wve_cols, axis=1)
    wvl_f = np.concatenate(wvl_cols, axis=1)
    wvx_t = wvx_f.astype(bf).reshape(HC, 128, 8).transpose(1, 0, 2).copy()
    wve_t = wve_f.astype(bf).reshape(HC, 128, 10).transpose(1, 0, 2).copy()
    wvl_t = wvl_f.astype(bf).reshape(HC, 128, 2).transpose(1, 0, 2).copy()
    linw_t = np.asarray(lin_w, np.float32).astype(bf).copy()
    linb_t = np.asarray(lin_b, np.float32).reshape(HC, 128).T.copy()
    ident = np.eye(128, dtype=bf)

    common = dict(XT=XT_t, Xn=Xn_t, theta_t=th_t, Wcat=Wcat_t, wvx=wvx_t, wve=wve_t,
                  wvl=wvl_t, linw=linw_t, linb=linb_t,
                  ones_row=np.ones((1, 128), np.float32),
                  ones_col=np.ones((128, 1), np.float32), ident=ident)
    maps = []
    pr = np.arange(128)
    for k in range(P):
        sel = np.zeros((128, RC, N), bf)
        for mi in range(RC):
            sel[pr, mi, k * R + mi * 128 + pr] = bf(-1e9)
        m = dict(common)
        m["XTk"] = XT_t[:, :, k * R:(k + 1) * R].copy()
        m["selbig"] = sel
        m["roff_in"] = np.array([[k * R]], np.uint32)
        maps.append(m)
    return maps


def kernel(**inputs):
    global _BUILT
    if _BUILT is None:
        _BUILT = build()
    maps = _prep_inputs(**inputs)
    res = run_bass_kernel_spmd(_BUILT, maps, core_ids=list(range(P)))
    outT = res.results[0]["out"]
    return np.ascontiguousarray(np.asarray(outT).T).astype(np.float32)
